# revision 32
# baseline (speedup 1.0000x reference)
"""Trainium2 Bass kernel for GQA attention with RoPE (nn_Attention_21603685499660).

Shapes (hardcoded): x [2, 2048, 4096], H=32 Q heads, KVH=8 KV heads, HD=128.
Sharding over 8 NeuronCores: core c -> batch b = c//4, head-group g = c%4
(8 Q heads, 2 KV heads per core).  Each core computes a partial output
(its heads' attention output through its slice of wo); the host sums the
4 partials per batch.  No on-device collectives.

Per-core pipeline (all matmuls bf16 with f32 PSUM accumulation):
  1. QKV projection from host-pre-transposed x and weights.  Q/K are
     produced directly in transposed [HD, seq] layout; V in natural
     [seq, HD] layout.  RoPE applied entirely on DVE (de-interleaved
     head dims host-side; rotation sign folded into the sin table).
     Input DMAs are spread across three HW-DGE queues so the PE never
     starves during chunk 0: weights on the SP queue, x chunks on the
     GpSimd queue (chunk 0 split into blocks alternating GpSimd/Act for
     progressive availability), cos/sin on the Act queue.  Chunk j+1's
     x is prefetched in one descriptor while chunk j computes.
  2. Attention with scores computed transposed: ST[k,q] = K @ Q^T per
     (head, q piece, k tile).  Softmax without max subtraction; the
     mask is multiplicative exp(mask) applied post-exp in bf16.
     Adjacent k tiles are PAIRED into one PSUM tile (2 tiles for 512-
     wide q pieces, 4 for 256-wide) with a single ScalarE exp per
     group, halving/quartering ScalarE instruction overhead.  Dead
     (fully masked) columns of diagonal tiles hold stale-but-bounded
     PSUM scores whose exp the mask multiply zeroes; piece 0 computes
     QK full-width so no never-written PSUM is ever read.  The softmax
     denominator is accumulated as a running bf16 sum on DVE and
     broadcast by ONE TensorE matmul per (piece, head); the head
     output is evacuated+normalized in one DVE tensor_mul against a
     fast-reciprocal of that broadcast.
  3. Output projection po[q,n] += attnT[d,q]^T @ woT[d,n], emitted as
     PE filler interleaved into subsequent pieces' attention; wo is
     prefetched on the GpSimd queue at stage-2 start and mask tiles
     one piece ahead on the SP queue, so the filler never stalls on
     DMA.  In the final drain the PSUM evacuation casts alternate
     DVE/ScalarE and the output DMAs alternate HW-DGE queues.
     Partial outputs ship bf16 (summed f32 on host).
"""

from contextlib import ExitStack

import numpy as np
import ml_dtypes

import concourse.bass as bass
import concourse.tile as tile
from concourse import bacc, mybir
from concourse.bass_utils import run_bass_kernel_spmd

B, S, D = 2, 2048, 4096
H, KVH, HD = 32, 8, 128
N_CORES = 8
GROUPS = 4            # head groups (tensor-parallel dim); B * GROUPS = 8 cores
HL = H // GROUPS      # 8 local Q heads
KVL = KVH // GROUPS   # 2 local KV heads
FQK = HL + KVL        # 10 feature tiles of 128 (Q heads then K heads)
NJ = S // 512         # 4 seq chunks of 512 (stage-1 granularity)
NT = S // 128         # 16 seq tiles of 128
ND = D // 128         # 32 contraction tiles
BF = mybir.dt.bfloat16
F32 = mybir.dt.float32

# attention q pieces (start, width); tapered tail so the last pieces'
# output projection can overlap preceding pieces
PIECES = [(0, 512), (512, 512), (1024, 512), (1536, 256), (1792, 256)]

_BUILD_CACHE: dict = {}


def _build(mask_mode: str):
    """mask_mode: 'causal' | 'zero' | 'general'."""
    nc = bacc.Bacc("TRN2", target_bir_lowering=False, debug=False,
                   num_devices=N_CORES)

    xt_d = nc.dram_tensor("xt", [128, NJ, ND, 512], BF, kind="ExternalInput").ap()
    wqk_d = nc.dram_tensor("wqk", [FQK, 128, ND, 128], BF, kind="ExternalInput").ap()
    wv_d = nc.dram_tensor("wv", [128, ND, KVL * HD], BF, kind="ExternalInput").ap()
    wo_d = nc.dram_tensor("wo", [128, HL, D], BF, kind="ExternalInput").ap()
    cos_d = nc.dram_tensor("cosd", [128, S], F32, kind="ExternalInput").ap()
    sin_d = nc.dram_tensor("sind", [128, S], F32, kind="ExternalInput").ap()
    if mask_mode == "causal":
        mk_d = nc.dram_tensor("maskd", [NJ, 4, 128, 512], BF, kind="ExternalInput").ap()
    elif mask_mode == "general":
        mk_d = nc.dram_tensor("maskt", [S, S], BF, kind="ExternalInput").ap()
    # partial outputs ship bf16 (host sums in f32): halves the 33.5MB/core
    # output DMA; the ~0.4% partial-sum rounding is small against the 2e-2
    # budget
    po_d = nc.dram_tensor("po", [S, D], BF, kind="ExternalOutput").ap()

    with tile.TileContext(nc) as tc, ExitStack() as ctx:
        resident = ctx.enter_context(tc.tile_pool(name="resident", bufs=1))
        qkv = ctx.enter_context(tc.tile_pool(name="qkv", bufs=1))

        ones128 = resident.tile([128, 128], BF)
        nc.vector.memset(ones128[:], 1.0)

        QT = qkv.tile([128, HL, S], BF)    # [HD, head, seq] (de-interleaved rows)
        KT = qkv.tile([128, KVL, S], BF)
        V = qkv.tile([128, NT, KVL * HD], BF)  # [seq%128, seqtile, kv-head*HD]

        # ---- stage 1: QKV projection + RoPE ----
        with tc.tile_pool(name="s1const", bufs=1) as s1const, \
             tc.tile_pool(name="xpool", bufs=2) as xpool, \
             tc.tile_pool(name="wpool", bufs=4) as wpool, \
             tc.tile_pool(name="tpool", bufs=3) as tpool, \
             tc.tile_pool(name="ps_qk", bufs=3, space="PSUM") as ps_qk, \
             tc.tile_pool(name="ps_w", bufs=2, space="PSUM") as ps_w, \
             tc.tile_pool(name="ps_v", bufs=2, space="PSUM") as ps_v:
            cosb = s1const.tile([128, S], F32)
            sinb = s1const.tile([128, S], F32)  # [-sin; +sin] halves
            wvb = s1const.tile([128, ND, KVL * HD], BF)
            # PE warm-up: dense ones@ones matmuls (no DMA dependency) keep
            # TensorE busy through the HAM window while the first x/weight
            # DMAs land, so real matmuls start at full clock.  Sized to end
            # just as chunk 0's first x block lands (idle would demote the
            # clock; excess would delay real work).
            for _ in range(48):
                wtile = ps_w.tile([128, 128], F32, tag="warm")
                nc.tensor.matmul(wtile[:], ones128[:], ones128[:],
                                 start=True, stop=True)

            def rope_emit(raw, f, js):
                # o = raw*cos + halfswap(raw)*sinN with no TensorE: the
                # half-swap is two partition-shifted ScalarE copies (same
                # engine as the evacuation, so ordering is free) and the
                # rotation sign lives in sinb = [-sin; +sin].
                rot = tpool.tile([128, 512], BF, tag="rot")
                nc.scalar.copy(out=rot[0:64, :], in_=raw[64:128, :])
                nc.scalar.copy(out=rot[64:128, :], in_=raw[0:64, :])
                t1 = tpool.tile([128, 512], F32, tag="t1")
                nc.vector.tensor_mul(t1[:], raw[:], cosb[:, js])
                t2 = tpool.tile([128, 512], F32, tag="t2")
                nc.vector.tensor_mul(t2[:], rot[:], sinb[:, js])
                dest = QT[:, f, js] if f < HL else KT[:, f - HL, js]
                nc.vector.tensor_add(dest, t1[:], t2[:])

            # The bulk x/cos/sin/wv stream rides the SP queue: the Sync
            # engine has no other stage-1 duties, so it can afford to stall
            # on DGE ring backpressure from many queued descriptors.
            # Weights ride the Act queue instead — never more than the
            # 3-deep prefetch window in flight, so ScalarE (which also runs
            # the PSUM evacuations) never blocks on a full ring.  GpSimd
            # issues instructions far too slowly to drive a queue.  Chunk 0
            # goes in 4-ktile blocks for progressive availability; chunks
            # 1-3 are single descriptors prefetched one chunk ahead.
            xtiles: dict = {}

            def xfetch(j):
                if j >= NJ or j in xtiles:
                    return
                xj = xpool.tile([128, ND, 512], BF, tag="x")
                if j == 0:
                    # first half on SP; back half via the otherwise-idle
                    # GpSimd queue (its slow issue rate still beats queueing
                    # 4.2MB behind one queue), so f0 is never x-starved
                    for blk in range(8):
                        eng = nc.sync if blk < 4 else nc.gpsimd
                        eng.dma_start(out=xj[:, 4 * blk:4 * blk + 4, :],
                                      in_=xt_d[:, j, 4 * blk:4 * blk + 4, :])
                else:
                    nc.sync.dma_start(out=xj[:], in_=xt_d[:, j])
                xtiles[j] = xj

            # weight prefetch runs a few tiles deep on its own (SP) queue
            n_groups = NJ * FQK
            wtiles: dict = {}
            wissued = 0

            def wprefetch(upto):
                nonlocal wissued
                while wissued < min(n_groups, upto):
                    wt = wpool.tile([128, ND, 128], BF, tag="wf")
                    nc.scalar.dma_start(out=wt[:], in_=wqk_d[wissued % FQK])
                    wtiles[wissued] = wt
                    wissued += 1

            xfetch(0)
            wprefetch(3)
            for j in range(NJ):
                js = bass.ts(j, 512)
                xj = xtiles.pop(j)
                for f in range(FQK):
                    gi = j * FQK + f
                    wf = wtiles.pop(gi)
                    # depth-4 window: the issue instruction sits in ScalarE's
                    # in-order stream ~one feature behind real time (behind
                    # evacuation waits), so a shallower window lets the PE
                    # catch up with the weight stream mid-chunk
                    wprefetch(gi + 4)
                    if j == 0 and f == 0:
                        # cos/sin full tensors behind chunk 0's x on the SP
                        # queue (a late sin only delays DVE-side rope, never
                        # the PE)
                        nc.sync.dma_start(out=cosb[:], in_=cos_d)
                        nc.sync.dma_start(out=sinb[:], in_=sin_d)
                    if j == 0 and f == 1:
                        nc.sync.dma_start(out=wvb[:], in_=wv_d[:])
                    if f == 4:
                        # prefetch next chunk's x while this chunk computes
                        # (behind wvb on the GpSimd queue for chunk 0)
                        xfetch(j + 1)
                    ps = ps_qk.tile([128, 512], F32, tag="qk")
                    for n in range(ND):
                        nc.tensor.matmul(ps[:], wf[:, n, :], xj[:, n, :],
                                         start=(n == 0), stop=(n == ND - 1))
                    raw = tpool.tile([128, 512], BF, tag="raw")
                    nc.scalar.copy(out=raw[:], in_=ps[:])
                    rope_emit(raw, f, js)
                for tt in range(4):
                    psv = ps_v.tile([128, KVL * HD], F32, tag="v")
                    for n in range(ND):
                        nc.tensor.matmul(psv[:], xj[:, n, bass.ts(tt, 128)],
                                         wvb[:, n, :],
                                         start=(n == 0), stop=(n == ND - 1))
                    nc.scalar.copy(out=V[:, j * 4 + tt, :], in_=psv[:])

        # attnT + wo live from stage 2 through stage 3 (pool opened only now
        # so stage 1 had the SBUF).
        att_out = ctx.enter_context(tc.tile_pool(name="att_out", bufs=1))
        attnT = att_out.tile([128, HL, S], BF)  # [HD, head, seq]
        wob = att_out.tile([128, HL, D], BF)

        # ---- stage 2+3: attention with interleaved output projection ----
        po_state = {"cur": None, "dd": 0, "drain": False, "alt": False,
                    "calt": False}

        def piece_atiles(pidx):
            q0, w = PIECES[pidx]
            if mask_mode == "zero":
                return q0, w, NT, []
            if mask_mode == "causal":
                nkt = (q0 + w) // 128
                return q0, w, nkt, list(range(q0 // 128, nkt))
            return q0, w, NT, list(range(NT))

        with tc.tile_pool(name="mpool", bufs=2 if mask_mode != "general" else 1) as mpool:
            pending_po = []  # (qt, nn) groups ready to emit as PE filler

            msk_tiles: dict = {}

            def load_msk(p):
                # mask tiles for piece p on the SP queue (issued one piece
                # ahead so they never gate a piece's first tensor_mul)
                if p >= len(PIECES) or p in msk_tiles:
                    return
                q0, w, nkt, atiles = piece_atiles(p)
                if not atiles:
                    msk_tiles[p] = None
                    return
                m = mpool.tile([128, len(atiles), w], BF, tag="msk")
                for idx, t in enumerate(atiles):
                    if mask_mode == "causal":
                        jj = t // 4
                        nc.sync.dma_start(
                            out=m[:, idx, :],
                            in_=mk_d[jj, t % 4][:, bass.ds(q0 - 512 * jj, w)])
                    else:
                        nc.sync.dma_start(
                            out=m[:, idx, :],
                            in_=mk_d[bass.ts(t, 128), bass.ds(q0, w)])
                msk_tiles[p] = m

            def run_piece(pidx, pools, pstep):
                ps_st, ps_o, ps_l, ppool, qpool, npool = pools
                q0, w, nkt, atiles = piece_atiles(pidx)
                js = bass.ds(q0, w)
                load_msk(pidx)  # no-op unless general mode (bufs=1, no prefetch)
                msk = msk_tiles.pop(pidx)
                if mask_mode != "general":
                    load_msk(pidx + 1)
                # k tiles are processed in PSUM-paired groups with one exp
                # per group — but only once po filler exists (pieces 0-1 are
                # latency-bound: coarser exp granularity exposes pipeline
                # latency the filler would otherwise cover)
                npair_max = 1 if pidx < 2 else (2 if w > 256 else 4)

                for h in range(HL):
                    hk = h // (HL // KVL)
                    outp = ps_o.tile([128, w], F32, tag="out")
                    lp = ps_l.tile([128, w], F32, tag="l")
                    pts = []
                    qsum = None
                    # software pipeline: PV_t is emitted one tile after QK_t
                    # so a full QK + filler sits in the PE stream while exp_t
                    # runs.  Diagonal tiles contribute nothing to masked
                    # columns, so PV runs only on the live sub-range.
                    def emit_pv(t):
                        off = max(0, 128 * t - q0) if mask_mode == "causal" else 0
                        nc.tensor.matmul(outp[:, off:w], V[:, t, bass.ts(hk, 128)],
                                         pts[t][:, off:w],
                                         start=(t == 0), stop=(t == nkt - 1),
                                         skip_group_check=True)

                    n_pv_done = 0
                    t = 0
                    while t < nkt:
                        npair = min(npair_max, nkt - t)
                        stp = ps_st.tile([128, npair, w], F32, tag="st")
                        pt2 = ppool.tile([128, npair, w], BF, tag="pt")
                        off0 = 0
                        for i in range(npair):
                            tt = t + i
                            # causal: columns q < 128t fully masked; compute
                            # QK only on the live sub-range.  Stale (finite)
                            # garbage in dead columns of paired tiles is
                            # exp'd then zeroed by the mask multiply (those
                            # PSUM slots have held bounded scores since the
                            # unpaired pieces 0-1 touched them full-width).
                            off = (max(0, 128 * tt - q0)
                                   if mask_mode == "causal" else 0)
                            if i == 0:
                                off0 = off
                            nc.tensor.matmul(stp[:, i, off:w],
                                             KT[:, hk, bass.ts(tt, 128)],
                                             QT[:, h, bass.ds(q0 + off, w - off)],
                                             start=True, stop=True)
                        if npair == 1 and off0 > 0:
                            # unpaired diagonal tile: exp only the live
                            # sub-range (never reads never-written PSUM)
                            nc.scalar.activation(
                                out=pt2[:, 0, off0:w], in_=stp[:, 0, off0:w],
                                func=mybir.ActivationFunctionType.Exp)
                        else:
                            nc.scalar.activation(
                                out=pt2[:], in_=stp[:],
                                func=mybir.ActivationFunctionType.Exp)
                        for i in range(npair):
                            tt = t + i
                            off_t = (max(0, 128 * tt - q0)
                                     if mask_mode == "causal" else 0)
                            if tt in atiles:
                                # multiplicative mask exp(m).  For causal
                                # masks only the 128-wide diagonal strip is
                                # partial — columns beyond it are all-ones
                                # and columns before it are skipped by every
                                # consumer, so the multiply (DVE) shrinks to
                                # the strip.
                                me = (min(off_t + 128, w)
                                      if mask_mode == "causal" else w)
                                mi = atiles.index(tt)
                                nc.vector.tensor_mul(
                                    pt2[:, i, off_t:me], pt2[:, i, off_t:me],
                                    msk[:, mi, off_t:me])
                            pts.append(pt2[:, i, :])
                            # running softmax-denominator sum in bf16 on DVE
                            # (live columns only); ONE broadcast matmul per
                            # (piece, head) at the end
                            if qsum is None:
                                qsum = qpool.tile([128, w], BF, tag="qs")
                                nc.vector.tensor_copy(qsum[:], pt2[:, i, :])
                            else:
                                nc.vector.tensor_add(qsum[:, off_t:w],
                                                     qsum[:, off_t:w],
                                                     pt2[:, i, off_t:w])
                        # wide pieces meter po filler so backlog survives
                        # into the tapered tail; narrow pieces drain harder
                        pstep({1: 1, 2: npair, 4: 6}[npair_max])
                        # PV lags one tile behind exp so a full QK + filler
                        # sits in the PE stream while exp runs
                        while n_pv_done < len(pts) - 1:
                            emit_pv(n_pv_done)
                            n_pv_done += 1
                        t += npair
                    nc.tensor.matmul(lp[:], ones128[:], qsum[:],
                                     start=True, stop=True)
                    while n_pv_done < nkt:
                        emit_pv(n_pv_done)
                        n_pv_done += 1
                    # fused evacuation + normalization on DVE (ScalarE stays
                    # exp-only; 1/l is a single fast-reciprocal op on the
                    # TensorE-broadcast denominator)
                    rcp = npool.tile([128, w], F32, tag="rcp")
                    nc.vector.reciprocal_approx_fast(out=rcp[:], in_=lp[:])
                    nc.vector.tensor_mul(attnT[:, h, js], outp[:], rcp[:])
                    # PE filler between heads covers the exp pipeline refill
                    pstep(16)
                pending_po.extend(
                    (qt, nn) for qt in range(q0 // 128, (q0 + w) // 128)
                    for nn in range(D // 512))

            def pstep_none(budget):
                return

            load_msk(0)
            if mask_mode != "general":  # bufs=1 pool: no prefetch
                load_msk(1)
            # wo heads 0-3 via the otherwise-idle GpSimd queue, heads 4-7
            # behind the first mask tiles on the SP queue; all land before
            # the po filler starts in piece 2.  Nothing rides the Act queue
            # here — its issuing engine (ScalarE) must stay free for exps.
            for dd in range(4):
                nc.gpsimd.dma_start(out=wob[:, dd, :], in_=wo_d[:, dd, :])
            for dd in range(4, HL):
                nc.sync.dma_start(out=wob[:, dd, :], in_=wo_d[:, dd, :])

            # block A — pieces 0-1, latency-bound, no filler available yet:
            # unpaired k tiles with a DEEP PSUM ring (4 score slots, double-
            # buffered output/denominator banks) so QK runs ahead of exp and
            # head boundaries never serialize on the evacuate chain.
            with tc.tile_pool(name="ppoolA", bufs=6) as ppoolA, \
                 tc.tile_pool(name="qpoolA", bufs=2) as qpoolA, \
                 tc.tile_pool(name="npoolA", bufs=2) as npoolA, \
                 tc.tile_pool(name="ps_stA", bufs=4, space="PSUM") as ps_stA, \
                 tc.tile_pool(name="ps_oA", bufs=2, space="PSUM") as ps_oA, \
                 tc.tile_pool(name="ps_lA", bufs=2, space="PSUM") as ps_lA:
                for pidx in (0, 1):
                    run_piece(pidx, (ps_stA, ps_oA, ps_lA, ppoolA, qpoolA,
                                     npoolA), pstep_none)

            # block B — pieces 2-4 + drain: paired exps + po filler
            with tc.tile_pool(name="ppool", bufs=8) as ppool, \
                 tc.tile_pool(name="qpool", bufs=2) as qpool, \
                 tc.tile_pool(name="npool", bufs=2) as npool, \
                 tc.tile_pool(name="spool", bufs=3) as spool, \
                 tc.tile_pool(name="ps_st", bufs=2, space="PSUM") as ps_st, \
                 tc.tile_pool(name="ps_o", bufs=1, space="PSUM") as ps_o, \
                 tc.tile_pool(name="ps_l", bufs=1, space="PSUM") as ps_l, \
                 tc.tile_pool(name="ps_po", bufs=2, space="PSUM") as ps_po:

                def po_step(budget):
                    # emit up to `budget` output-projection matmuls as PE
                    # filler; a group's PSUM accumulation legally interleaves
                    # with other banks' matmuls
                    for _ in range(budget):
                        if po_state["cur"] is None:
                            if not pending_po:
                                return
                            qt, nn = pending_po.pop(0)
                            pop = ps_po.tile([128, 512], F32, tag="po")
                            po_state["cur"] = (qt, nn, pop)
                            po_state["dd"] = 0
                        qt, nn, pop = po_state["cur"]
                        dd = po_state["dd"]
                        nc.tensor.matmul(pop[:], attnT[:, dd, bass.ts(qt, 128)],
                                         wob[:, dd, bass.ts(nn, 512)],
                                         start=(dd == 0), stop=(dd == HL - 1))
                        po_state["dd"] += 1
                        if po_state["dd"] == HL:
                            stg = spool.tile([128, 512], BF, tag="stg")
                            # in the drain ScalarE is exp-free: alternate the
                            # PSUM evacuation casts between DVE and ScalarE,
                            # and the output DMAs between the SP and Act
                            # HW-DGE queues, so neither tail-chains after the
                            # last matmuls
                            if po_state["drain"] and po_state["calt"]:
                                nc.scalar.copy(out=stg[:], in_=pop[:])
                            else:
                                nc.vector.tensor_copy(stg[:], pop[:])
                            po_state["calt"] = not po_state["calt"]
                            eng = nc.scalar if (po_state["drain"] and
                                                po_state["alt"]) else nc.sync
                            po_state["alt"] = not po_state["alt"]
                            eng.dma_start(
                                out=po_d[bass.ts(qt, 128), bass.ts(nn, 512)],
                                in_=stg[:])
                            po_state["cur"] = None

                for pidx in (2, 3, 4):
                    if pidx == len(PIECES) - 1:
                        po_state["drain"] = True
                    run_piece(pidx, (ps_st, ps_o, ps_l, ppool, qpool, npool),
                              po_step)
                po_state["drain"] = True
                while pending_po or po_state["cur"] is not None:
                    po_step(8)

    nc.compile()
    return nc


def _get_nc(mask_mode: str):
    if mask_mode not in _BUILD_CACHE:
        _BUILD_CACHE[mask_mode] = _build(mask_mode)
    return _BUILD_CACHE[mask_mode]


_DEINT = np.concatenate([np.arange(0, HD, 2), np.arange(1, HD, 2)])  # de-interleave


def _host_prep(x, freqs_cos, freqs_sin, mask, wq, wk, wv, wo):
    bf16 = ml_dtypes.bfloat16
    scale = float(HD) ** -0.5

    # mask mode
    mask = np.asarray(mask, np.float32)
    tril = np.tril(np.ones((S, S), bool))
    if np.all(mask == 0):
        mask_mode = "zero"
    elif np.all(mask[tril] == 0) and np.all(mask[~tril] <= -1e8):
        mask_mode = "causal"
    else:
        mask_mode = "general"

    # weights: de-interleave head dims of wq/wk; fold softmax scale into wq
    wq_p = (np.asarray(wq, np.float32).reshape(H, HD, D)[:, _DEINT, :] * scale)
    wk_p = np.asarray(wk, np.float32).reshape(KVH, HD, D)[:, _DEINT, :]
    wv_n = np.asarray(wv, np.float32).reshape(KVH, HD, D)
    wo_n = np.asarray(wo, np.float32)

    per_group = []
    for g in range(GROUPS):
        feats = np.concatenate([
            wq_p[g * HL:(g + 1) * HL].reshape(HL * HD, D),
            wk_p[g * KVL:(g + 1) * KVL].reshape(KVL * HD, D),
        ], axis=0)  # [1280, D]
        wqk_dma = np.ascontiguousarray(
            feats.reshape(FQK, 128, ND, 128).transpose(0, 3, 2, 1)).astype(bf16)
        wvg = wv_n[g * KVL:(g + 1) * KVL].reshape(KVL * HD, D)
        wv_dma = np.ascontiguousarray(
            wvg.reshape(KVL * HD, ND, 128).transpose(2, 1, 0)).astype(bf16)
        woT = wo_n[:, g * HL * HD:(g + 1) * HL * HD].T  # [1024, D]
        wo_dma = np.ascontiguousarray(
            woT.reshape(HL, 128, D).transpose(1, 0, 2)).astype(bf16)
        per_group.append((wqk_dma, wv_dma, wo_dma))

    xs = []
    for b in range(B):
        xT = np.asarray(x[b], np.float32).T  # [D, S]
        xs.append(np.ascontiguousarray(
            xT.reshape(ND, 128, NJ, 512).transpose(1, 2, 0, 3)).astype(bf16))

    cosT = np.asarray(freqs_cos, np.float32).T  # [64, S]
    sinT = np.asarray(freqs_sin, np.float32).T
    cos_dma = np.ascontiguousarray(np.concatenate([cosT, cosT], 0))
    # rotation sign folded into the sin table: o = raw*cos + halfswap(raw)*sinN
    sin_dma = np.ascontiguousarray(np.concatenate([-sinT, sinT], 0))

    # mask is applied multiplicatively after exp: P *= exp(mask)
    mask_extra = {}
    if mask_mode == "causal":
        mT = np.exp(np.minimum(mask.T, 0.0))
        md = np.empty((NJ, 4, 128, 512), np.float32)
        for j in range(NJ):
            for i in range(4):
                t = 4 * j + i
                md[j, i] = mT[t * 128:(t + 1) * 128, j * 512:(j + 1) * 512]
        mask_extra["maskd"] = md.astype(bf16)
    elif mask_mode == "general":
        with np.errstate(over="ignore"):
            mask_extra["maskt"] = np.ascontiguousarray(
                np.exp(mask.T)).astype(bf16)

    in_maps = []
    for c in range(N_CORES):
        b, g = c // GROUPS, c % GROUPS
        wqk_dma, wv_dma, wo_dma = per_group[g]
        m = {"xt": xs[b], "wqk": wqk_dma, "wv": wv_dma, "wo": wo_dma,
             "cosd": cos_dma, "sind": sin_dma}
        m.update(mask_extra)
        in_maps.append(m)
    return mask_mode, in_maps


def kernel(x, freqs_cos, freqs_sin, positions, mask, wq, wk, wv, wo,
           _want_profile=False):
    mask_mode, in_maps = _host_prep(x, freqs_cos, freqs_sin, mask, wq, wk, wv, wo)
    nc = _get_nc(mask_mode)
    res = run_bass_kernel_spmd(nc, in_maps, core_ids=list(range(N_CORES)),
                               trace=_want_profile)
    out = np.zeros((B, S, D), np.float32)
    for c in range(N_CORES):
        out[c // GROUPS] += np.asarray(res.results[c]["po"], np.float32)
    if _want_profile:
        kernel.last_exec_time_ns = res.exec_time_ns
        kernel.last_results = res
    return out


# revision 33
# speedup vs baseline: 1.0151x; 1.0151x over previous
"""Trainium2 Bass kernel for GQA attention with RoPE (nn_Attention_21603685499660).

Shapes (hardcoded): x [2, 2048, 4096], H=32 Q heads, KVH=8 KV heads, HD=128.
Sharding over 8 NeuronCores: core c -> batch b = c//4, head-group g = c%4
(8 Q heads, 2 KV heads per core).  Each core computes a partial output
(its heads' attention output through its slice of wo); the host sums the
4 partials per batch.  No on-device collectives.

Per-core pipeline (all matmuls bf16 with f32 PSUM accumulation):
  1. QKV projection from host-pre-transposed x and weights.  Q/K are
     produced directly in transposed [HD, seq] layout; V in natural
     [seq, HD] layout.  RoPE applied entirely on DVE (de-interleaved
     head dims host-side; rotation sign folded into the sin table).
     Input DMAs are spread across three HW-DGE queues so the PE never
     starves during chunk 0: weights on the SP queue, x chunks on the
     GpSimd queue (chunk 0 split into blocks alternating GpSimd/Act for
     progressive availability), cos/sin on the Act queue.  Chunk j+1's
     x is prefetched in one descriptor while chunk j computes.
  2. Attention with scores computed transposed: ST[k,q] = K @ Q^T per
     (head, q piece, k tile).  Softmax without max subtraction; the
     mask is multiplicative exp(mask) applied post-exp in bf16.
     Adjacent k tiles are PAIRED into one PSUM tile (2 tiles for 512-
     wide q pieces, 4 for 256-wide) with a single ScalarE exp per
     group, halving/quartering ScalarE instruction overhead.  Dead
     (fully masked) columns of diagonal tiles hold stale-but-bounded
     PSUM scores whose exp the mask multiply zeroes; piece 0 computes
     QK full-width so no never-written PSUM is ever read.  The softmax
     denominator is accumulated as a running bf16 sum on DVE and
     broadcast by ONE TensorE matmul per (piece, head); the head
     output is evacuated+normalized in one DVE tensor_mul against a
     fast-reciprocal of that broadcast.
  3. Output projection po[q,n] += attnT[d,q]^T @ woT[d,n], emitted as
     PE filler interleaved into subsequent pieces' attention; wo is
     prefetched on the GpSimd queue at stage-2 start and mask tiles
     one piece ahead on the SP queue, so the filler never stalls on
     DMA.  In the final drain the PSUM evacuation casts alternate
     DVE/ScalarE and the output DMAs alternate HW-DGE queues.
     Partial outputs ship bf16 (summed f32 on host).
"""

from contextlib import ExitStack

import numpy as np
import ml_dtypes

import concourse.bass as bass
import concourse.tile as tile
from concourse import bacc, mybir
from concourse.bass_utils import run_bass_kernel_spmd

B, S, D = 2, 2048, 4096
H, KVH, HD = 32, 8, 128
N_CORES = 8
GROUPS = 4            # head groups (tensor-parallel dim); B * GROUPS = 8 cores
HL = H // GROUPS      # 8 local Q heads
KVL = KVH // GROUPS   # 2 local KV heads
FQK = HL + KVL        # 10 feature tiles of 128 (Q heads then K heads)
NJ = S // 512         # 4 seq chunks of 512 (stage-1 granularity)
NT = S // 128         # 16 seq tiles of 128
ND = D // 128         # 32 contraction tiles
BF = mybir.dt.bfloat16
F32 = mybir.dt.float32

# attention q pieces (start, width); tapered tail so the last pieces'
# output projection can overlap preceding pieces
PIECES = [(0, 512), (512, 512), (1024, 512), (1536, 256), (1792, 256)]

_BUILD_CACHE: dict = {}


def _build(mask_mode: str):
    """mask_mode: 'causal' | 'zero' | 'general'."""
    nc = bacc.Bacc("TRN2", target_bir_lowering=False, debug=False,
                   num_devices=N_CORES)

    xt_d = nc.dram_tensor("xt", [128, NJ, ND, 512], BF, kind="ExternalInput").ap()
    wqk_d = nc.dram_tensor("wqk", [FQK, 128, ND, 128], BF, kind="ExternalInput").ap()
    wv_d = nc.dram_tensor("wv", [128, ND, KVL * HD], BF, kind="ExternalInput").ap()
    wo_d = nc.dram_tensor("wo", [128, HL, D], BF, kind="ExternalInput").ap()
    cos_d = nc.dram_tensor("cosd", [128, S], F32, kind="ExternalInput").ap()
    sin_d = nc.dram_tensor("sind", [128, S], F32, kind="ExternalInput").ap()
    if mask_mode == "causal":
        mk_d = nc.dram_tensor("maskd", [NJ, 4, 128, 512], BF, kind="ExternalInput").ap()
    elif mask_mode == "general":
        mk_d = nc.dram_tensor("maskt", [S, S], BF, kind="ExternalInput").ap()
    # partial outputs ship bf16 (host sums in f32): halves the 33.5MB/core
    # output DMA; the ~0.4% partial-sum rounding is small against the 2e-2
    # budget
    po_d = nc.dram_tensor("po", [S, D], BF, kind="ExternalOutput").ap()

    with tile.TileContext(nc) as tc, ExitStack() as ctx:
        resident = ctx.enter_context(tc.tile_pool(name="resident", bufs=1))
        qkv = ctx.enter_context(tc.tile_pool(name="qkv", bufs=1))

        ones128 = resident.tile([128, 128], BF)
        nc.vector.memset(ones128[:], 1.0)

        QT = qkv.tile([128, HL, S], BF)    # [HD, head, seq] (de-interleaved rows)
        KT = qkv.tile([128, KVL, S], BF)
        V = qkv.tile([128, NT, KVL * HD], BF)  # [seq%128, seqtile, kv-head*HD]

        # ---- stage 1: QKV projection + RoPE ----
        with tc.tile_pool(name="s1const", bufs=1) as s1const, \
             tc.tile_pool(name="xpool", bufs=2) as xpool, \
             tc.tile_pool(name="wpool", bufs=4) as wpool, \
             tc.tile_pool(name="tpool", bufs=3) as tpool, \
             tc.tile_pool(name="ps_qk", bufs=3, space="PSUM") as ps_qk, \
             tc.tile_pool(name="ps_w", bufs=2, space="PSUM") as ps_w, \
             tc.tile_pool(name="ps_v", bufs=2, space="PSUM") as ps_v:
            cosb = s1const.tile([128, S], F32)
            sinb = s1const.tile([128, S], F32)  # [-sin; +sin] halves
            wvb = s1const.tile([128, ND, KVL * HD], BF)
            # PE warm-up: dense ones@ones matmuls (no DMA dependency) keep
            # TensorE busy through the HAM window while the first x/weight
            # DMAs land, so real matmuls start at full clock.  Sized to end
            # just as chunk 0's first x block lands (idle would demote the
            # clock; excess would delay real work).
            for _ in range(48):
                wtile = ps_w.tile([128, 128], F32, tag="warm")
                nc.tensor.matmul(wtile[:], ones128[:], ones128[:],
                                 start=True, stop=True)

            def rope_emit(raw, f, js):
                # o = raw*cos + halfswap(raw)*sinN with no TensorE: the
                # half-swap is two partition-shifted ScalarE copies (same
                # engine as the evacuation, so ordering is free) and the
                # rotation sign lives in sinb = [-sin; +sin].
                rot = tpool.tile([128, 512], BF, tag="rot")
                nc.scalar.copy(out=rot[0:64, :], in_=raw[64:128, :])
                nc.scalar.copy(out=rot[64:128, :], in_=raw[0:64, :])
                t1 = tpool.tile([128, 512], F32, tag="t1")
                nc.vector.tensor_mul(t1[:], raw[:], cosb[:, js])
                t2 = tpool.tile([128, 512], F32, tag="t2")
                nc.vector.tensor_mul(t2[:], rot[:], sinb[:, js])
                dest = QT[:, f, js] if f < HL else KT[:, f - HL, js]
                nc.vector.tensor_add(dest, t1[:], t2[:])

            # The bulk x/cos/sin/wv stream rides the SP queue: the Sync
            # engine has no other stage-1 duties, so it can afford to stall
            # on DGE ring backpressure from many queued descriptors.
            # Weights ride the Act queue instead — never more than the
            # 3-deep prefetch window in flight, so ScalarE (which also runs
            # the PSUM evacuations) never blocks on a full ring.  GpSimd
            # issues instructions far too slowly to drive a queue.  Chunk 0
            # goes in 4-ktile blocks for progressive availability; chunks
            # 1-3 are single descriptors prefetched one chunk ahead.
            xtiles: dict = {}

            def xfetch(j):
                if j >= NJ or j in xtiles:
                    return
                xj = xpool.tile([128, ND, 512], BF, tag="x")
                if j == 0:
                    for blk in range(8):
                        nc.sync.dma_start(out=xj[:, 4 * blk:4 * blk + 4, :],
                                          in_=xt_d[:, j, 4 * blk:4 * blk + 4, :])
                else:
                    nc.sync.dma_start(out=xj[:], in_=xt_d[:, j])
                xtiles[j] = xj

            # weight prefetch runs a few tiles deep on its own (SP) queue
            n_groups = NJ * FQK
            wtiles: dict = {}
            wissued = 0

            def wprefetch(upto):
                nonlocal wissued
                while wissued < min(n_groups, upto):
                    wt = wpool.tile([128, ND, 128], BF, tag="wf")
                    nc.scalar.dma_start(out=wt[:], in_=wqk_d[wissued % FQK])
                    wtiles[wissued] = wt
                    wissued += 1

            xfetch(0)
            wprefetch(3)
            for j in range(NJ):
                js = bass.ts(j, 512)
                xj = xtiles.pop(j)
                for f in range(FQK):
                    gi = j * FQK + f
                    wf = wtiles.pop(gi)
                    # depth-4 window: the issue instruction sits in ScalarE's
                    # in-order stream ~one feature behind real time (behind
                    # evacuation waits), so a shallower window lets the PE
                    # catch up with the weight stream mid-chunk
                    wprefetch(gi + 4)
                    if j == 0 and f == 0:
                        # cos/sin full tensors behind chunk 0's x on the SP
                        # queue (a late sin only delays DVE-side rope, never
                        # the PE)
                        nc.sync.dma_start(out=cosb[:], in_=cos_d)
                        nc.sync.dma_start(out=sinb[:], in_=sin_d)
                    if j == 0 and f == 1:
                        nc.sync.dma_start(out=wvb[:], in_=wv_d[:])
                    if f == 4:
                        # prefetch next chunk's x while this chunk computes
                        # (behind wvb on the GpSimd queue for chunk 0)
                        xfetch(j + 1)
                    ps = ps_qk.tile([128, 512], F32, tag="qk")
                    for n in range(ND):
                        nc.tensor.matmul(ps[:], wf[:, n, :], xj[:, n, :],
                                         start=(n == 0), stop=(n == ND - 1))
                    raw = tpool.tile([128, 512], BF, tag="raw")
                    nc.scalar.copy(out=raw[:], in_=ps[:])
                    rope_emit(raw, f, js)
                for tt in range(4):
                    psv = ps_v.tile([128, KVL * HD], F32, tag="v")
                    for n in range(ND):
                        nc.tensor.matmul(psv[:], xj[:, n, bass.ts(tt, 128)],
                                         wvb[:, n, :],
                                         start=(n == 0), stop=(n == ND - 1))
                    nc.scalar.copy(out=V[:, j * 4 + tt, :], in_=psv[:])

        # attnT + wo live from stage 2 through stage 3 (pool opened only now
        # so stage 1 had the SBUF).
        att_out = ctx.enter_context(tc.tile_pool(name="att_out", bufs=1))
        attnT = att_out.tile([128, HL, S], BF)  # [HD, head, seq]
        wob = att_out.tile([128, HL, D], BF)

        # ---- stage 2+3: attention with interleaved output projection ----
        po_state = {"cur": None, "dd": 0, "drain": False, "alt": False,
                    "calt": False}

        def piece_atiles(pidx):
            q0, w = PIECES[pidx]
            if mask_mode == "zero":
                return q0, w, NT, []
            if mask_mode == "causal":
                nkt = (q0 + w) // 128
                return q0, w, nkt, list(range(q0 // 128, nkt))
            return q0, w, NT, list(range(NT))

        with tc.tile_pool(name="mpool", bufs=2 if mask_mode != "general" else 1) as mpool:
            pending_po = []  # (qt, nn) groups ready to emit as PE filler

            msk_tiles: dict = {}

            def load_msk(p):
                # mask tiles for piece p on the SP queue (issued one piece
                # ahead so they never gate a piece's first tensor_mul)
                if p >= len(PIECES) or p in msk_tiles:
                    return
                q0, w, nkt, atiles = piece_atiles(p)
                if not atiles:
                    msk_tiles[p] = None
                    return
                m = mpool.tile([128, len(atiles), w], BF, tag="msk")
                for idx, t in enumerate(atiles):
                    if mask_mode == "causal":
                        jj = t // 4
                        nc.sync.dma_start(
                            out=m[:, idx, :],
                            in_=mk_d[jj, t % 4][:, bass.ds(q0 - 512 * jj, w)])
                    else:
                        nc.sync.dma_start(
                            out=m[:, idx, :],
                            in_=mk_d[bass.ts(t, 128), bass.ds(q0, w)])
                msk_tiles[p] = m

            def run_piece(pidx, pools, pstep):
                ps_st, ps_o, ps_l, ppool, qpool, npool = pools
                q0, w, nkt, atiles = piece_atiles(pidx)
                js = bass.ds(q0, w)
                load_msk(pidx)  # no-op unless general mode (bufs=1, no prefetch)
                msk = msk_tiles.pop(pidx)
                if mask_mode != "general":
                    load_msk(pidx + 1)
                # k tiles are processed in PSUM-paired groups with one exp
                # per group — but only once po filler exists (pieces 0-1 are
                # latency-bound: coarser exp granularity exposes pipeline
                # latency the filler would otherwise cover)
                npair_max = 1 if pidx < 2 else (2 if w > 256 else 4)

                for h in range(HL):
                    hk = h // (HL // KVL)
                    outp = ps_o.tile([128, w], F32, tag="out")
                    lp = ps_l.tile([128, w], F32, tag="l")
                    pts = []
                    qsum = None
                    # software pipeline: PV_t is emitted one tile after QK_t
                    # so a full QK + filler sits in the PE stream while exp_t
                    # runs.  Diagonal tiles contribute nothing to masked
                    # columns, so PV runs only on the live sub-range.
                    def emit_pv(t):
                        off = max(0, 128 * t - q0) if mask_mode == "causal" else 0
                        nc.tensor.matmul(outp[:, off:w], V[:, t, bass.ts(hk, 128)],
                                         pts[t][:, off:w],
                                         start=(t == 0), stop=(t == nkt - 1),
                                         skip_group_check=True)

                    n_pv_done = 0
                    t = 0
                    while t < nkt:
                        npair = min(npair_max, nkt - t)
                        stp = ps_st.tile([128, npair, w], F32, tag="st")
                        pt2 = ppool.tile([128, npair, w], BF, tag="pt")
                        off0 = 0
                        for i in range(npair):
                            tt = t + i
                            # causal: columns q < 128t fully masked; compute
                            # QK only on the live sub-range.  Stale (finite)
                            # garbage in dead columns of paired tiles is
                            # exp'd then zeroed by the mask multiply (those
                            # PSUM slots have held bounded scores since the
                            # unpaired pieces 0-1 touched them full-width).
                            off = (max(0, 128 * tt - q0)
                                   if mask_mode == "causal" else 0)
                            if i == 0:
                                off0 = off
                            nc.tensor.matmul(stp[:, i, off:w],
                                             KT[:, hk, bass.ts(tt, 128)],
                                             QT[:, h, bass.ds(q0 + off, w - off)],
                                             start=True, stop=True)
                        if npair == 1 and off0 > 0:
                            # unpaired diagonal tile: exp only the live
                            # sub-range (never reads never-written PSUM)
                            nc.scalar.activation(
                                out=pt2[:, 0, off0:w], in_=stp[:, 0, off0:w],
                                func=mybir.ActivationFunctionType.Exp)
                        else:
                            nc.scalar.activation(
                                out=pt2[:], in_=stp[:],
                                func=mybir.ActivationFunctionType.Exp)
                        for i in range(npair):
                            tt = t + i
                            off_t = (max(0, 128 * tt - q0)
                                     if mask_mode == "causal" else 0)
                            if tt in atiles:
                                # multiplicative mask exp(m).  For causal
                                # masks only the 128-wide diagonal strip is
                                # partial — columns beyond it are all-ones
                                # and columns before it are skipped by every
                                # consumer, so the multiply (DVE) shrinks to
                                # the strip.
                                me = (min(off_t + 128, w)
                                      if mask_mode == "causal" else w)
                                mi = atiles.index(tt)
                                nc.vector.tensor_mul(
                                    pt2[:, i, off_t:me], pt2[:, i, off_t:me],
                                    msk[:, mi, off_t:me])
                            pts.append(pt2[:, i, :])
                            # running softmax-denominator sum in bf16 on DVE
                            # (live columns only); ONE broadcast matmul per
                            # (piece, head) at the end
                            if qsum is None:
                                qsum = qpool.tile([128, w], BF, tag="qs")
                                nc.vector.tensor_copy(qsum[:], pt2[:, i, :])
                            else:
                                nc.vector.tensor_add(qsum[:, off_t:w],
                                                     qsum[:, off_t:w],
                                                     pt2[:, i, off_t:w])
                        # wide pieces meter po filler so backlog survives
                        # into the tapered tail; narrow pieces drain harder
                        pstep({1: 1, 2: npair, 4: 6}[npair_max])
                        # PV lags one tile behind exp so a full QK + filler
                        # sits in the PE stream while exp runs
                        while n_pv_done < len(pts) - 1:
                            emit_pv(n_pv_done)
                            n_pv_done += 1
                        t += npair
                    nc.tensor.matmul(lp[:], ones128[:], qsum[:],
                                     start=True, stop=True)
                    while n_pv_done < nkt:
                        emit_pv(n_pv_done)
                        n_pv_done += 1
                    # fused evacuation + normalization on DVE (ScalarE stays
                    # exp-only; 1/l is a single fast-reciprocal op on the
                    # TensorE-broadcast denominator)
                    rcp = npool.tile([128, w], F32, tag="rcp")
                    nc.vector.reciprocal_approx_fast(out=rcp[:], in_=lp[:])
                    nc.vector.tensor_mul(attnT[:, h, js], outp[:], rcp[:])
                    # PE filler between heads covers the exp pipeline refill
                    pstep(16)
                pending_po.extend(
                    (qt, nn) for qt in range(q0 // 128, (q0 + w) // 128)
                    for nn in range(D // 512))

            def pstep_none(budget):
                return

            load_msk(0)
            if mask_mode != "general":  # bufs=1 pool: no prefetch
                load_msk(1)
            # wo heads 0-3 via the otherwise-idle GpSimd queue, heads 4-7
            # behind the first mask tiles on the SP queue; all land before
            # the po filler starts in piece 2.  Nothing rides the Act queue
            # here — its issuing engine (ScalarE) must stay free for exps.
            for dd in range(4):
                nc.gpsimd.dma_start(out=wob[:, dd, :], in_=wo_d[:, dd, :])
            for dd in range(4, HL):
                nc.sync.dma_start(out=wob[:, dd, :], in_=wo_d[:, dd, :])

            # block A — pieces 0-1, latency-bound, no filler available yet:
            # unpaired k tiles with a DEEP PSUM ring (4 score slots, double-
            # buffered output/denominator banks) so QK runs ahead of exp and
            # head boundaries never serialize on the evacuate chain.
            with tc.tile_pool(name="ppoolA", bufs=6) as ppoolA, \
                 tc.tile_pool(name="qpoolA", bufs=2) as qpoolA, \
                 tc.tile_pool(name="npoolA", bufs=2) as npoolA, \
                 tc.tile_pool(name="ps_stA", bufs=4, space="PSUM") as ps_stA, \
                 tc.tile_pool(name="ps_oA", bufs=2, space="PSUM") as ps_oA, \
                 tc.tile_pool(name="ps_lA", bufs=2, space="PSUM") as ps_lA:
                for pidx in (0, 1):
                    run_piece(pidx, (ps_stA, ps_oA, ps_lA, ppoolA, qpoolA,
                                     npoolA), pstep_none)

            # block B — pieces 2-4 + drain: paired exps + po filler
            with tc.tile_pool(name="ppool", bufs=8) as ppool, \
                 tc.tile_pool(name="qpool", bufs=2) as qpool, \
                 tc.tile_pool(name="npool", bufs=2) as npool, \
                 tc.tile_pool(name="spool", bufs=3) as spool, \
                 tc.tile_pool(name="ps_st", bufs=2, space="PSUM") as ps_st, \
                 tc.tile_pool(name="ps_o", bufs=1, space="PSUM") as ps_o, \
                 tc.tile_pool(name="ps_l", bufs=1, space="PSUM") as ps_l, \
                 tc.tile_pool(name="ps_po", bufs=2, space="PSUM") as ps_po:

                def po_step(budget):
                    # emit up to `budget` output-projection matmuls as PE
                    # filler; a group's PSUM accumulation legally interleaves
                    # with other banks' matmuls
                    for _ in range(budget):
                        if po_state["cur"] is None:
                            if not pending_po:
                                return
                            qt, nn = pending_po.pop(0)
                            pop = ps_po.tile([128, 512], F32, tag="po")
                            po_state["cur"] = (qt, nn, pop)
                            po_state["dd"] = 0
                        qt, nn, pop = po_state["cur"]
                        dd = po_state["dd"]
                        nc.tensor.matmul(pop[:], attnT[:, dd, bass.ts(qt, 128)],
                                         wob[:, dd, bass.ts(nn, 512)],
                                         start=(dd == 0), stop=(dd == HL - 1))
                        po_state["dd"] += 1
                        if po_state["dd"] == HL:
                            stg = spool.tile([128, 512], BF, tag="stg")
                            # in the drain ScalarE is exp-free: alternate the
                            # PSUM evacuation casts between DVE and ScalarE,
                            # and the output DMAs between the SP and Act
                            # HW-DGE queues, so neither tail-chains after the
                            # last matmuls
                            if po_state["drain"] and po_state["calt"]:
                                nc.scalar.copy(out=stg[:], in_=pop[:])
                            else:
                                nc.vector.tensor_copy(stg[:], pop[:])
                            po_state["calt"] = not po_state["calt"]
                            eng = nc.scalar if (po_state["drain"] and
                                                po_state["alt"]) else nc.sync
                            po_state["alt"] = not po_state["alt"]
                            eng.dma_start(
                                out=po_d[bass.ts(qt, 128), bass.ts(nn, 512)],
                                in_=stg[:])
                            po_state["cur"] = None

                for pidx in (2, 3, 4):
                    if pidx == len(PIECES) - 1:
                        po_state["drain"] = True
                    run_piece(pidx, (ps_st, ps_o, ps_l, ppool, qpool, npool),
                              po_step)
                po_state["drain"] = True
                while pending_po or po_state["cur"] is not None:
                    po_step(8)

    nc.compile()
    return nc


def _get_nc(mask_mode: str):
    if mask_mode not in _BUILD_CACHE:
        _BUILD_CACHE[mask_mode] = _build(mask_mode)
    return _BUILD_CACHE[mask_mode]


_DEINT = np.concatenate([np.arange(0, HD, 2), np.arange(1, HD, 2)])  # de-interleave


def _host_prep(x, freqs_cos, freqs_sin, mask, wq, wk, wv, wo):
    bf16 = ml_dtypes.bfloat16
    scale = float(HD) ** -0.5

    # mask mode
    mask = np.asarray(mask, np.float32)
    tril = np.tril(np.ones((S, S), bool))
    if np.all(mask == 0):
        mask_mode = "zero"
    elif np.all(mask[tril] == 0) and np.all(mask[~tril] <= -1e8):
        mask_mode = "causal"
    else:
        mask_mode = "general"

    # weights: de-interleave head dims of wq/wk; fold softmax scale into wq
    wq_p = (np.asarray(wq, np.float32).reshape(H, HD, D)[:, _DEINT, :] * scale)
    wk_p = np.asarray(wk, np.float32).reshape(KVH, HD, D)[:, _DEINT, :]
    wv_n = np.asarray(wv, np.float32).reshape(KVH, HD, D)
    wo_n = np.asarray(wo, np.float32)

    per_group = []
    for g in range(GROUPS):
        feats = np.concatenate([
            wq_p[g * HL:(g + 1) * HL].reshape(HL * HD, D),
            wk_p[g * KVL:(g + 1) * KVL].reshape(KVL * HD, D),
        ], axis=0)  # [1280, D]
        wqk_dma = np.ascontiguousarray(
            feats.reshape(FQK, 128, ND, 128).transpose(0, 3, 2, 1)).astype(bf16)
        wvg = wv_n[g * KVL:(g + 1) * KVL].reshape(KVL * HD, D)
        wv_dma = np.ascontiguousarray(
            wvg.reshape(KVL * HD, ND, 128).transpose(2, 1, 0)).astype(bf16)
        woT = wo_n[:, g * HL * HD:(g + 1) * HL * HD].T  # [1024, D]
        wo_dma = np.ascontiguousarray(
            woT.reshape(HL, 128, D).transpose(1, 0, 2)).astype(bf16)
        per_group.append((wqk_dma, wv_dma, wo_dma))

    xs = []
    for b in range(B):
        xT = np.asarray(x[b], np.float32).T  # [D, S]
        xs.append(np.ascontiguousarray(
            xT.reshape(ND, 128, NJ, 512).transpose(1, 2, 0, 3)).astype(bf16))

    cosT = np.asarray(freqs_cos, np.float32).T  # [64, S]
    sinT = np.asarray(freqs_sin, np.float32).T
    cos_dma = np.ascontiguousarray(np.concatenate([cosT, cosT], 0))
    # rotation sign folded into the sin table: o = raw*cos + halfswap(raw)*sinN
    sin_dma = np.ascontiguousarray(np.concatenate([-sinT, sinT], 0))

    # mask is applied multiplicatively after exp: P *= exp(mask)
    mask_extra = {}
    if mask_mode == "causal":
        mT = np.exp(np.minimum(mask.T, 0.0))
        md = np.empty((NJ, 4, 128, 512), np.float32)
        for j in range(NJ):
            for i in range(4):
                t = 4 * j + i
                md[j, i] = mT[t * 128:(t + 1) * 128, j * 512:(j + 1) * 512]
        mask_extra["maskd"] = md.astype(bf16)
    elif mask_mode == "general":
        with np.errstate(over="ignore"):
            mask_extra["maskt"] = np.ascontiguousarray(
                np.exp(mask.T)).astype(bf16)

    in_maps = []
    for c in range(N_CORES):
        b, g = c // GROUPS, c % GROUPS
        wqk_dma, wv_dma, wo_dma = per_group[g]
        m = {"xt": xs[b], "wqk": wqk_dma, "wv": wv_dma, "wo": wo_dma,
             "cosd": cos_dma, "sind": sin_dma}
        m.update(mask_extra)
        in_maps.append(m)
    return mask_mode, in_maps


def kernel(x, freqs_cos, freqs_sin, positions, mask, wq, wk, wv, wo,
           _want_profile=False):
    mask_mode, in_maps = _host_prep(x, freqs_cos, freqs_sin, mask, wq, wk, wv, wo)
    nc = _get_nc(mask_mode)
    res = run_bass_kernel_spmd(nc, in_maps, core_ids=list(range(N_CORES)),
                               trace=_want_profile)
    out = np.zeros((B, S, D), np.float32)
    for c in range(N_CORES):
        out[c // GROUPS] += np.asarray(res.results[c]["po"], np.float32)
    if _want_profile:
        kernel.last_exec_time_ns = res.exec_time_ns
        kernel.last_results = res
    return out


# revision 36
# speedup vs baseline: 1.0153x; 1.0002x over previous
"""Trainium2 Bass kernel for GQA attention with RoPE (nn_Attention_21603685499660).

Shapes (hardcoded): x [2, 2048, 4096], H=32 Q heads, KVH=8 KV heads, HD=128.
Sharding over 8 NeuronCores: core c -> batch b = c//4, head-group g = c%4
(8 Q heads, 2 KV heads per core).  Each core computes a partial output
(its heads' attention output through its slice of wo); the host sums the
4 partials per batch.  No on-device collectives.

Per-core pipeline (all matmuls bf16 with f32 PSUM accumulation):
  1. QKV projection from host-pre-transposed x and weights.  Q/K are
     produced directly in transposed [HD, seq] layout; V in natural
     [seq, HD] layout.  RoPE applied entirely on DVE (de-interleaved
     head dims host-side; rotation sign folded into the sin table).
     Input DMAs are spread across three HW-DGE queues so the PE never
     starves during chunk 0: weights on the SP queue, x chunks on the
     GpSimd queue (chunk 0 split into blocks alternating GpSimd/Act for
     progressive availability), cos/sin on the Act queue.  Chunk j+1's
     x is prefetched in one descriptor while chunk j computes.
  2. Attention with scores computed transposed: ST[k,q] = K @ Q^T per
     (head, q piece, k tile).  Softmax without max subtraction; the
     mask is multiplicative exp(mask) applied post-exp in bf16.
     Adjacent k tiles are PAIRED into one PSUM tile (2 tiles for 512-
     wide q pieces, 4 for 256-wide) with a single ScalarE exp per
     group, halving/quartering ScalarE instruction overhead.  Dead
     (fully masked) columns of diagonal tiles hold stale-but-bounded
     PSUM scores whose exp the mask multiply zeroes; piece 0 computes
     QK full-width so no never-written PSUM is ever read.  The softmax
     denominator is accumulated as a running bf16 sum on DVE and
     broadcast by ONE TensorE matmul per (piece, head); the head
     output is evacuated+normalized in one DVE tensor_mul against a
     fast-reciprocal of that broadcast.
  3. Output projection po[q,n] += attnT[d,q]^T @ woT[d,n], emitted as
     PE filler interleaved into subsequent pieces' attention; wo is
     prefetched on the GpSimd queue at stage-2 start and mask tiles
     one piece ahead on the SP queue, so the filler never stalls on
     DMA.  In the final drain the PSUM evacuation casts alternate
     DVE/ScalarE and the output DMAs alternate HW-DGE queues.
     Partial outputs ship bf16 (summed f32 on host).
"""

from contextlib import ExitStack

import numpy as np
import ml_dtypes

import concourse.bass as bass
import concourse.tile as tile
from concourse import bacc, mybir
from concourse.bass_utils import run_bass_kernel_spmd

B, S, D = 2, 2048, 4096
H, KVH, HD = 32, 8, 128
N_CORES = 8
GROUPS = 4            # head groups (tensor-parallel dim); B * GROUPS = 8 cores
HL = H // GROUPS      # 8 local Q heads
KVL = KVH // GROUPS   # 2 local KV heads
FQK = HL + KVL        # 10 feature tiles of 128 (Q heads then K heads)
NJ = S // 512         # 4 seq chunks of 512 (stage-1 granularity)
NT = S // 128         # 16 seq tiles of 128
ND = D // 128         # 32 contraction tiles
BF = mybir.dt.bfloat16
F32 = mybir.dt.float32

# attention q pieces (start, width); tapered tail so the last pieces'
# output projection can overlap preceding pieces
PIECES = [(0, 512), (512, 512), (1024, 512), (1536, 256), (1792, 256)]

_BUILD_CACHE: dict = {}


def _build(mask_mode: str):
    """mask_mode: 'causal' | 'zero' | 'general'."""
    nc = bacc.Bacc("TRN2", target_bir_lowering=False, debug=False,
                   num_devices=N_CORES)

    xt_d = nc.dram_tensor("xt", [128, NJ, ND, 512], BF, kind="ExternalInput").ap()
    wqk_d = nc.dram_tensor("wqk", [FQK, 128, ND, 128], BF, kind="ExternalInput").ap()
    wv_d = nc.dram_tensor("wv", [128, ND, KVL * HD], BF, kind="ExternalInput").ap()
    wo_d = nc.dram_tensor("wo", [128, HL, D], BF, kind="ExternalInput").ap()
    cos_d = nc.dram_tensor("cosd", [128, S], F32, kind="ExternalInput").ap()
    sin_d = nc.dram_tensor("sind", [128, S], F32, kind="ExternalInput").ap()
    if mask_mode == "causal":
        mk_d = nc.dram_tensor("maskd", [NJ, 4, 128, 512], BF, kind="ExternalInput").ap()
    elif mask_mode == "general":
        mk_d = nc.dram_tensor("maskt", [S, S], BF, kind="ExternalInput").ap()
    # partial outputs ship bf16 (host sums in f32): halves the 33.5MB/core
    # output DMA; the ~0.4% partial-sum rounding is small against the 2e-2
    # budget
    po_d = nc.dram_tensor("po", [S, D], BF, kind="ExternalOutput").ap()

    with tile.TileContext(nc) as tc, ExitStack() as ctx:
        resident = ctx.enter_context(tc.tile_pool(name="resident", bufs=1))
        qkv = ctx.enter_context(tc.tile_pool(name="qkv", bufs=1))

        ones128 = resident.tile([128, 128], BF)
        nc.vector.memset(ones128[:], 1.0)

        QT = qkv.tile([128, HL, S], BF)    # [HD, head, seq] (de-interleaved rows)
        KT = qkv.tile([128, KVL, S], BF)
        V = qkv.tile([128, NT, KVL * HD], BF)  # [seq%128, seqtile, kv-head*HD]

        # ---- stage 1: QKV projection + RoPE ----
        with tc.tile_pool(name="s1const", bufs=1) as s1const, \
             tc.tile_pool(name="xpool", bufs=2) as xpool, \
             tc.tile_pool(name="wpool", bufs=4) as wpool, \
             tc.tile_pool(name="tpool", bufs=3) as tpool, \
             tc.tile_pool(name="ps_qk", bufs=3, space="PSUM") as ps_qk, \
             tc.tile_pool(name="ps_w", bufs=2, space="PSUM") as ps_w, \
             tc.tile_pool(name="ps_v", bufs=2, space="PSUM") as ps_v:
            cosb = s1const.tile([128, S], F32)
            sinb = s1const.tile([128, S], F32)  # [-sin; +sin] halves
            wvb = s1const.tile([128, ND, KVL * HD], BF)
            # PE warm-up: dense ones@ones matmuls (no DMA dependency) keep
            # TensorE busy through the HAM window while the first x/weight
            # DMAs land, so real matmuls start at full clock.  Sized to end
            # just as chunk 0's first x block lands (idle would demote the
            # clock; excess would delay real work).
            for _ in range(24):
                wtile = ps_w.tile([128, 128], F32, tag="warm")
                nc.tensor.matmul(wtile[:], ones128[:], ones128[:],
                                 start=True, stop=True)

            def rope_emit(raw, f, js):
                # o = raw*cos + halfswap(raw)*sinN with no TensorE: the
                # half-swap is two partition-shifted ScalarE copies (same
                # engine as the evacuation, so ordering is free) and the
                # rotation sign lives in sinb = [-sin; +sin].
                rot = tpool.tile([128, 512], BF, tag="rot")
                nc.scalar.copy(out=rot[0:64, :], in_=raw[64:128, :])
                nc.scalar.copy(out=rot[64:128, :], in_=raw[0:64, :])
                t1 = tpool.tile([128, 512], F32, tag="t1")
                nc.vector.tensor_mul(t1[:], raw[:], cosb[:, js])
                t2 = tpool.tile([128, 512], F32, tag="t2")
                nc.vector.tensor_mul(t2[:], rot[:], sinb[:, js])
                dest = QT[:, f, js] if f < HL else KT[:, f - HL, js]
                nc.vector.tensor_add(dest, t1[:], t2[:])

            # The bulk x/cos/sin/wv stream rides the SP queue: the Sync
            # engine has no other stage-1 duties, so it can afford to stall
            # on DGE ring backpressure from many queued descriptors.
            # Weights ride the Act queue instead — never more than the
            # 3-deep prefetch window in flight, so ScalarE (which also runs
            # the PSUM evacuations) never blocks on a full ring.  GpSimd
            # issues instructions far too slowly to drive a queue.  Chunk 0
            # goes in 4-ktile blocks for progressive availability; chunks
            # 1-3 are single descriptors prefetched one chunk ahead.
            xtiles: dict = {}

            def xfetch(j):
                if j >= NJ or j in xtiles:
                    return
                xj = xpool.tile([128, ND, 512], BF, tag="x")
                if j == 0:
                    for blk in range(8):
                        nc.sync.dma_start(out=xj[:, 4 * blk:4 * blk + 4, :],
                                          in_=xt_d[:, j, 4 * blk:4 * blk + 4, :])
                else:
                    nc.sync.dma_start(out=xj[:], in_=xt_d[:, j])
                xtiles[j] = xj

            # weight prefetch runs a few tiles deep on its own (SP) queue
            n_groups = NJ * FQK
            wtiles: dict = {}
            wissued = 0

            def wprefetch(upto):
                nonlocal wissued
                while wissued < min(n_groups, upto):
                    wt = wpool.tile([128, ND, 128], BF, tag="wf")
                    # w0 rides the SP queue as its very first descriptor:
                    # the Act queue ramps slowly while SP hogs early HBM
                    # bandwidth, and w0 gates the first real matmul
                    eng = nc.sync if wissued == 0 else nc.scalar
                    eng.dma_start(out=wt[:], in_=wqk_d[wissued % FQK])
                    wtiles[wissued] = wt
                    wissued += 1

            wprefetch(3)  # w0 leads the SP queue; then chunk 0's x blocks
            xfetch(0)
            for j in range(NJ):
                js = bass.ts(j, 512)
                xj = xtiles.pop(j)
                for f in range(FQK):
                    gi = j * FQK + f
                    wf = wtiles.pop(gi)
                    # depth-4 window: the issue instruction sits in ScalarE's
                    # in-order stream ~one feature behind real time (behind
                    # evacuation waits), so a shallower window lets the PE
                    # catch up with the weight stream mid-chunk
                    wprefetch(gi + 4)
                    if j == 0 and f == 0:
                        # cos/sin full tensors behind chunk 0's x on the SP
                        # queue (a late sin only delays DVE-side rope, never
                        # the PE)
                        nc.sync.dma_start(out=cosb[:], in_=cos_d)
                        nc.sync.dma_start(out=sinb[:], in_=sin_d)
                    if j == 0 and f == 1:
                        nc.sync.dma_start(out=wvb[:], in_=wv_d[:])
                    if f == 4:
                        # prefetch next chunk's x while this chunk computes
                        # (behind wvb on the GpSimd queue for chunk 0)
                        xfetch(j + 1)
                    ps = ps_qk.tile([128, 512], F32, tag="qk")
                    for n in range(ND):
                        nc.tensor.matmul(ps[:], wf[:, n, :], xj[:, n, :],
                                         start=(n == 0), stop=(n == ND - 1))
                    raw = tpool.tile([128, 512], BF, tag="raw")
                    nc.scalar.copy(out=raw[:], in_=ps[:])
                    rope_emit(raw, f, js)
                for tt in range(4):
                    psv = ps_v.tile([128, KVL * HD], F32, tag="v")
                    for n in range(ND):
                        nc.tensor.matmul(psv[:], xj[:, n, bass.ts(tt, 128)],
                                         wvb[:, n, :],
                                         start=(n == 0), stop=(n == ND - 1))
                    nc.scalar.copy(out=V[:, j * 4 + tt, :], in_=psv[:])

        # attnT + wo live from stage 2 through stage 3 (pool opened only now
        # so stage 1 had the SBUF).
        att_out = ctx.enter_context(tc.tile_pool(name="att_out", bufs=1))
        attnT = att_out.tile([128, HL, S], BF)  # [HD, head, seq]
        wob = att_out.tile([128, HL, D], BF)

        # ---- stage 2+3: attention with interleaved output projection ----
        po_state = {"cur": None, "dd": 0, "drain": False, "alt": False,
                    "calt": False}

        def piece_atiles(pidx):
            q0, w = PIECES[pidx]
            if mask_mode == "zero":
                return q0, w, NT, []
            if mask_mode == "causal":
                nkt = (q0 + w) // 128
                return q0, w, nkt, list(range(q0 // 128, nkt))
            return q0, w, NT, list(range(NT))

        with tc.tile_pool(name="mpool", bufs=2 if mask_mode != "general" else 1) as mpool:
            pending_po = []  # (qt, nn) groups ready to emit as PE filler

            msk_tiles: dict = {}

            def load_msk(p):
                # mask tiles for piece p on the SP queue (issued one piece
                # ahead so they never gate a piece's first tensor_mul)
                if p >= len(PIECES) or p in msk_tiles:
                    return
                q0, w, nkt, atiles = piece_atiles(p)
                if not atiles:
                    msk_tiles[p] = None
                    return
                m = mpool.tile([128, len(atiles), w], BF, tag="msk")
                for idx, t in enumerate(atiles):
                    if mask_mode == "causal":
                        jj = t // 4
                        nc.sync.dma_start(
                            out=m[:, idx, :],
                            in_=mk_d[jj, t % 4][:, bass.ds(q0 - 512 * jj, w)])
                    else:
                        nc.sync.dma_start(
                            out=m[:, idx, :],
                            in_=mk_d[bass.ts(t, 128), bass.ds(q0, w)])
                msk_tiles[p] = m

            def run_piece(pidx, pools, pstep):
                ps_st, ps_o, ps_l, ppool, qpool, npool = pools
                q0, w, nkt, atiles = piece_atiles(pidx)
                js = bass.ds(q0, w)
                load_msk(pidx)  # no-op unless general mode (bufs=1, no prefetch)
                msk = msk_tiles.pop(pidx)
                if mask_mode != "general":
                    load_msk(pidx + 1)
                # k tiles are processed in PSUM-paired groups with one exp
                # per group — but only once po filler exists (pieces 0-1 are
                # latency-bound: coarser exp granularity exposes pipeline
                # latency the filler would otherwise cover)
                npair_max = 1 if pidx < 2 else (2 if w > 256 else 4)

                for h in range(HL):
                    hk = h // (HL // KVL)
                    outp = ps_o.tile([128, w], F32, tag="out")
                    lp = ps_l.tile([128, w], F32, tag="l")
                    pts = []
                    qsum = None
                    # software pipeline: PV_t is emitted one tile after QK_t
                    # so a full QK + filler sits in the PE stream while exp_t
                    # runs.  Diagonal tiles contribute nothing to masked
                    # columns, so PV runs only on the live sub-range.
                    def emit_pv(t):
                        off = max(0, 128 * t - q0) if mask_mode == "causal" else 0
                        nc.tensor.matmul(outp[:, off:w], V[:, t, bass.ts(hk, 128)],
                                         pts[t][:, off:w],
                                         start=(t == 0), stop=(t == nkt - 1),
                                         skip_group_check=True)

                    n_pv_done = 0
                    t = 0
                    while t < nkt:
                        npair = min(npair_max, nkt - t)
                        stp = ps_st.tile([128, npair, w], F32, tag="st")
                        pt2 = ppool.tile([128, npair, w], BF, tag="pt")
                        off0 = 0
                        for i in range(npair):
                            tt = t + i
                            # causal: columns q < 128t fully masked; compute
                            # QK only on the live sub-range.  Stale (finite)
                            # garbage in dead columns of paired tiles is
                            # exp'd then zeroed by the mask multiply (those
                            # PSUM slots have held bounded scores since the
                            # unpaired pieces 0-1 touched them full-width).
                            off = (max(0, 128 * tt - q0)
                                   if mask_mode == "causal" else 0)
                            if i == 0:
                                off0 = off
                            nc.tensor.matmul(stp[:, i, off:w],
                                             KT[:, hk, bass.ts(tt, 128)],
                                             QT[:, h, bass.ds(q0 + off, w - off)],
                                             start=True, stop=True)
                        if npair == 1 and off0 > 0:
                            # unpaired diagonal tile: exp only the live
                            # sub-range (never reads never-written PSUM)
                            nc.scalar.activation(
                                out=pt2[:, 0, off0:w], in_=stp[:, 0, off0:w],
                                func=mybir.ActivationFunctionType.Exp)
                        else:
                            nc.scalar.activation(
                                out=pt2[:], in_=stp[:],
                                func=mybir.ActivationFunctionType.Exp)
                        for i in range(npair):
                            tt = t + i
                            off_t = (max(0, 128 * tt - q0)
                                     if mask_mode == "causal" else 0)
                            if tt in atiles:
                                # multiplicative mask exp(m).  For causal
                                # masks only the 128-wide diagonal strip is
                                # partial — columns beyond it are all-ones
                                # and columns before it are skipped by every
                                # consumer, so the multiply (DVE) shrinks to
                                # the strip.
                                me = (min(off_t + 128, w)
                                      if mask_mode == "causal" else w)
                                mi = atiles.index(tt)
                                nc.vector.tensor_mul(
                                    pt2[:, i, off_t:me], pt2[:, i, off_t:me],
                                    msk[:, mi, off_t:me])
                            pts.append(pt2[:, i, :])
                            # running softmax-denominator sum in bf16 on DVE
                            # (live columns only); ONE broadcast matmul per
                            # (piece, head) at the end
                            if qsum is None:
                                qsum = qpool.tile([128, w], BF, tag="qs")
                                nc.vector.tensor_copy(qsum[:], pt2[:, i, :])
                            else:
                                nc.vector.tensor_add(qsum[:, off_t:w],
                                                     qsum[:, off_t:w],
                                                     pt2[:, i, off_t:w])
                        # wide pieces meter po filler so backlog survives
                        # into the tapered tail; narrow pieces drain harder
                        pstep({1: 1, 2: npair, 4: 6}[npair_max])
                        # PV lags one tile behind exp so a full QK + filler
                        # sits in the PE stream while exp runs
                        while n_pv_done < len(pts) - 1:
                            emit_pv(n_pv_done)
                            n_pv_done += 1
                        t += npair
                    nc.tensor.matmul(lp[:], ones128[:], qsum[:],
                                     start=True, stop=True)
                    while n_pv_done < nkt:
                        emit_pv(n_pv_done)
                        n_pv_done += 1
                    # fused evacuation + normalization on DVE (ScalarE stays
                    # exp-only; 1/l is a single fast-reciprocal op on the
                    # TensorE-broadcast denominator)
                    rcp = npool.tile([128, w], F32, tag="rcp")
                    nc.vector.reciprocal_approx_fast(out=rcp[:], in_=lp[:])
                    nc.vector.tensor_mul(attnT[:, h, js], outp[:], rcp[:])
                    # PE filler between heads covers the exp pipeline refill
                    pstep(16)
                pending_po.extend(
                    (qt, nn) for qt in range(q0 // 128, (q0 + w) // 128)
                    for nn in range(D // 512))

            def pstep_none(budget):
                return

            load_msk(0)
            if mask_mode != "general":  # bufs=1 pool: no prefetch
                load_msk(1)
            # wo heads 0-3 via the otherwise-idle GpSimd queue, heads 4-7
            # behind the first mask tiles on the SP queue; all land before
            # the po filler starts in piece 2.  Nothing rides the Act queue
            # here — its issuing engine (ScalarE) must stay free for exps.
            for dd in range(4):
                nc.gpsimd.dma_start(out=wob[:, dd, :], in_=wo_d[:, dd, :])
            for dd in range(4, HL):
                nc.sync.dma_start(out=wob[:, dd, :], in_=wo_d[:, dd, :])

            # block A — pieces 0-1, latency-bound, no filler available yet:
            # unpaired k tiles with a DEEP PSUM ring (4 score slots, double-
            # buffered output/denominator banks) so QK runs ahead of exp and
            # head boundaries never serialize on the evacuate chain.
            with tc.tile_pool(name="ppoolA", bufs=6) as ppoolA, \
                 tc.tile_pool(name="qpoolA", bufs=2) as qpoolA, \
                 tc.tile_pool(name="npoolA", bufs=2) as npoolA, \
                 tc.tile_pool(name="ps_stA", bufs=4, space="PSUM") as ps_stA, \
                 tc.tile_pool(name="ps_oA", bufs=2, space="PSUM") as ps_oA, \
                 tc.tile_pool(name="ps_lA", bufs=2, space="PSUM") as ps_lA:
                for pidx in (0, 1):
                    run_piece(pidx, (ps_stA, ps_oA, ps_lA, ppoolA, qpoolA,
                                     npoolA), pstep_none)

            # block B — pieces 2-4 + drain: paired exps + po filler
            with tc.tile_pool(name="ppool", bufs=8) as ppool, \
                 tc.tile_pool(name="qpool", bufs=2) as qpool, \
                 tc.tile_pool(name="npool", bufs=2) as npool, \
                 tc.tile_pool(name="spool", bufs=3) as spool, \
                 tc.tile_pool(name="ps_st", bufs=2, space="PSUM") as ps_st, \
                 tc.tile_pool(name="ps_o", bufs=1, space="PSUM") as ps_o, \
                 tc.tile_pool(name="ps_l", bufs=1, space="PSUM") as ps_l, \
                 tc.tile_pool(name="ps_po", bufs=2, space="PSUM") as ps_po:

                def po_step(budget):
                    # emit up to `budget` output-projection matmuls as PE
                    # filler; a group's PSUM accumulation legally interleaves
                    # with other banks' matmuls
                    for _ in range(budget):
                        if po_state["cur"] is None:
                            if not pending_po:
                                return
                            qt, nn = pending_po.pop(0)
                            pop = ps_po.tile([128, 512], F32, tag="po")
                            po_state["cur"] = (qt, nn, pop)
                            po_state["dd"] = 0
                        qt, nn, pop = po_state["cur"]
                        dd = po_state["dd"]
                        nc.tensor.matmul(pop[:], attnT[:, dd, bass.ts(qt, 128)],
                                         wob[:, dd, bass.ts(nn, 512)],
                                         start=(dd == 0), stop=(dd == HL - 1))
                        po_state["dd"] += 1
                        if po_state["dd"] == HL:
                            stg = spool.tile([128, 512], BF, tag="stg")
                            # in the drain ScalarE is exp-free: alternate the
                            # PSUM evacuation casts between DVE and ScalarE,
                            # and the output DMAs between the SP and Act
                            # HW-DGE queues, so neither tail-chains after the
                            # last matmuls
                            if po_state["drain"] and po_state["calt"]:
                                nc.scalar.copy(out=stg[:], in_=pop[:])
                            else:
                                nc.vector.tensor_copy(stg[:], pop[:])
                            po_state["calt"] = not po_state["calt"]
                            eng = nc.scalar if (po_state["drain"] and
                                                po_state["alt"]) else nc.sync
                            po_state["alt"] = not po_state["alt"]
                            eng.dma_start(
                                out=po_d[bass.ts(qt, 128), bass.ts(nn, 512)],
                                in_=stg[:])
                            po_state["cur"] = None

                for pidx in (2, 3, 4):
                    if pidx == len(PIECES) - 1:
                        po_state["drain"] = True
                    run_piece(pidx, (ps_st, ps_o, ps_l, ppool, qpool, npool),
                              po_step)
                po_state["drain"] = True
                while pending_po or po_state["cur"] is not None:
                    po_step(8)

    nc.compile()
    return nc


def _get_nc(mask_mode: str):
    if mask_mode not in _BUILD_CACHE:
        _BUILD_CACHE[mask_mode] = _build(mask_mode)
    return _BUILD_CACHE[mask_mode]


_DEINT = np.concatenate([np.arange(0, HD, 2), np.arange(1, HD, 2)])  # de-interleave


def _host_prep(x, freqs_cos, freqs_sin, mask, wq, wk, wv, wo):
    bf16 = ml_dtypes.bfloat16
    scale = float(HD) ** -0.5

    # mask mode
    mask = np.asarray(mask, np.float32)
    tril = np.tril(np.ones((S, S), bool))
    if np.all(mask == 0):
        mask_mode = "zero"
    elif np.all(mask[tril] == 0) and np.all(mask[~tril] <= -1e8):
        mask_mode = "causal"
    else:
        mask_mode = "general"

    # weights: de-interleave head dims of wq/wk; fold softmax scale into wq
    wq_p = (np.asarray(wq, np.float32).reshape(H, HD, D)[:, _DEINT, :] * scale)
    wk_p = np.asarray(wk, np.float32).reshape(KVH, HD, D)[:, _DEINT, :]
    wv_n = np.asarray(wv, np.float32).reshape(KVH, HD, D)
    wo_n = np.asarray(wo, np.float32)

    per_group = []
    for g in range(GROUPS):
        feats = np.concatenate([
            wq_p[g * HL:(g + 1) * HL].reshape(HL * HD, D),
            wk_p[g * KVL:(g + 1) * KVL].reshape(KVL * HD, D),
        ], axis=0)  # [1280, D]
        wqk_dma = np.ascontiguousarray(
            feats.reshape(FQK, 128, ND, 128).transpose(0, 3, 2, 1)).astype(bf16)
        wvg = wv_n[g * KVL:(g + 1) * KVL].reshape(KVL * HD, D)
        wv_dma = np.ascontiguousarray(
            wvg.reshape(KVL * HD, ND, 128).transpose(2, 1, 0)).astype(bf16)
        woT = wo_n[:, g * HL * HD:(g + 1) * HL * HD].T  # [1024, D]
        wo_dma = np.ascontiguousarray(
            woT.reshape(HL, 128, D).transpose(1, 0, 2)).astype(bf16)
        per_group.append((wqk_dma, wv_dma, wo_dma))

    xs = []
    for b in range(B):
        xT = np.asarray(x[b], np.float32).T  # [D, S]
        xs.append(np.ascontiguousarray(
            xT.reshape(ND, 128, NJ, 512).transpose(1, 2, 0, 3)).astype(bf16))

    cosT = np.asarray(freqs_cos, np.float32).T  # [64, S]
    sinT = np.asarray(freqs_sin, np.float32).T
    cos_dma = np.ascontiguousarray(np.concatenate([cosT, cosT], 0))
    # rotation sign folded into the sin table: o = raw*cos + halfswap(raw)*sinN
    sin_dma = np.ascontiguousarray(np.concatenate([-sinT, sinT], 0))

    # mask is applied multiplicatively after exp: P *= exp(mask)
    mask_extra = {}
    if mask_mode == "causal":
        mT = np.exp(np.minimum(mask.T, 0.0))
        md = np.empty((NJ, 4, 128, 512), np.float32)
        for j in range(NJ):
            for i in range(4):
                t = 4 * j + i
                md[j, i] = mT[t * 128:(t + 1) * 128, j * 512:(j + 1) * 512]
        mask_extra["maskd"] = md.astype(bf16)
    elif mask_mode == "general":
        with np.errstate(over="ignore"):
            mask_extra["maskt"] = np.ascontiguousarray(
                np.exp(mask.T)).astype(bf16)

    in_maps = []
    for c in range(N_CORES):
        b, g = c // GROUPS, c % GROUPS
        wqk_dma, wv_dma, wo_dma = per_group[g]
        m = {"xt": xs[b], "wqk": wqk_dma, "wv": wv_dma, "wo": wo_dma,
             "cosd": cos_dma, "sind": sin_dma}
        m.update(mask_extra)
        in_maps.append(m)
    return mask_mode, in_maps


def kernel(x, freqs_cos, freqs_sin, positions, mask, wq, wk, wv, wo,
           _want_profile=False):
    mask_mode, in_maps = _host_prep(x, freqs_cos, freqs_sin, mask, wq, wk, wv, wo)
    nc = _get_nc(mask_mode)
    res = run_bass_kernel_spmd(nc, in_maps, core_ids=list(range(N_CORES)),
                               trace=_want_profile)
    out = np.zeros((B, S, D), np.float32)
    for c in range(N_CORES):
        out[c // GROUPS] += np.asarray(res.results[c]["po"], np.float32)
    if _want_profile:
        kernel.last_exec_time_ns = res.exec_time_ns
        kernel.last_results = res
    return out


# revision 37
# speedup vs baseline: 1.0173x; 1.0019x over previous
"""Trainium2 Bass kernel for GQA attention with RoPE (nn_Attention_21603685499660).

Shapes (hardcoded): x [2, 2048, 4096], H=32 Q heads, KVH=8 KV heads, HD=128.
Sharding over 8 NeuronCores: core c -> batch b = c//4, head-group g = c%4
(8 Q heads, 2 KV heads per core).  Each core computes a partial output
(its heads' attention output through its slice of wo); the host sums the
4 partials per batch.  No on-device collectives.

Per-core pipeline (all matmuls bf16 with f32 PSUM accumulation):
  1. QKV projection from host-pre-transposed x and weights.  Q/K are
     produced directly in transposed [HD, seq] layout; V in natural
     [seq, HD] layout.  RoPE applied entirely on DVE (de-interleaved
     head dims host-side; rotation sign folded into the sin table).
     Input DMAs are spread across three HW-DGE queues so the PE never
     starves during chunk 0: weights on the SP queue, x chunks on the
     GpSimd queue (chunk 0 split into blocks alternating GpSimd/Act for
     progressive availability), cos/sin on the Act queue.  Chunk j+1's
     x is prefetched in one descriptor while chunk j computes.
  2. Attention with scores computed transposed: ST[k,q] = K @ Q^T per
     (head, q piece, k tile).  Softmax without max subtraction; the
     mask is multiplicative exp(mask) applied post-exp in bf16.
     Adjacent k tiles are PAIRED into one PSUM tile (2 tiles for 512-
     wide q pieces, 4 for 256-wide) with a single ScalarE exp per
     group, halving/quartering ScalarE instruction overhead.  Dead
     (fully masked) columns of diagonal tiles hold stale-but-bounded
     PSUM scores whose exp the mask multiply zeroes; piece 0 computes
     QK full-width so no never-written PSUM is ever read.  The softmax
     denominator is accumulated as a running bf16 sum on DVE and
     broadcast by ONE TensorE matmul per (piece, head); the head
     output is evacuated+normalized in one DVE tensor_mul against a
     fast-reciprocal of that broadcast.
  3. Output projection po[q,n] += attnT[d,q]^T @ woT[d,n], emitted as
     PE filler interleaved into subsequent pieces' attention; wo is
     prefetched on the GpSimd queue at stage-2 start and mask tiles
     one piece ahead on the SP queue, so the filler never stalls on
     DMA.  In the final drain the PSUM evacuation casts alternate
     DVE/ScalarE and the output DMAs alternate HW-DGE queues.
     Partial outputs ship bf16 (summed f32 on host).
"""

from contextlib import ExitStack

import numpy as np
import ml_dtypes

import concourse.bass as bass
import concourse.tile as tile
from concourse import bacc, mybir
from concourse.bass_utils import run_bass_kernel_spmd

B, S, D = 2, 2048, 4096
H, KVH, HD = 32, 8, 128
N_CORES = 8
GROUPS = 4            # head groups (tensor-parallel dim); B * GROUPS = 8 cores
HL = H // GROUPS      # 8 local Q heads
KVL = KVH // GROUPS   # 2 local KV heads
FQK = HL + KVL        # 10 feature tiles of 128 (Q heads then K heads)
NJ = S // 512         # 4 seq chunks of 512 (stage-1 granularity)
NT = S // 128         # 16 seq tiles of 128
ND = D // 128         # 32 contraction tiles
BF = mybir.dt.bfloat16
F32 = mybir.dt.float32

# attention q pieces (start, width); tapered tail so the last pieces'
# output projection can overlap preceding pieces
PIECES = [(0, 512), (512, 512), (1024, 512), (1536, 256), (1792, 256)]

_BUILD_CACHE: dict = {}


def _build(mask_mode: str):
    """mask_mode: 'causal' | 'zero' | 'general'."""
    nc = bacc.Bacc("TRN2", target_bir_lowering=False, debug=False,
                   num_devices=N_CORES)

    xt_d = nc.dram_tensor("xt", [128, NJ, ND, 512], BF, kind="ExternalInput").ap()
    wqk_d = nc.dram_tensor("wqk", [FQK, 128, ND, 128], BF, kind="ExternalInput").ap()
    wv_d = nc.dram_tensor("wv", [128, ND, KVL * HD], BF, kind="ExternalInput").ap()
    wo_d = nc.dram_tensor("wo", [128, HL, D], BF, kind="ExternalInput").ap()
    cos_d = nc.dram_tensor("cosd", [128, S], F32, kind="ExternalInput").ap()
    sin_d = nc.dram_tensor("sind", [128, S], F32, kind="ExternalInput").ap()
    if mask_mode == "causal":
        mk_d = nc.dram_tensor("maskd", [NJ, 4, 128, 512], BF, kind="ExternalInput").ap()
    elif mask_mode == "general":
        mk_d = nc.dram_tensor("maskt", [S, S], BF, kind="ExternalInput").ap()
    # partial outputs ship bf16 (host sums in f32): halves the 33.5MB/core
    # output DMA; the ~0.4% partial-sum rounding is small against the 2e-2
    # budget
    po_d = nc.dram_tensor("po", [S, D], BF, kind="ExternalOutput").ap()

    with tile.TileContext(nc) as tc, ExitStack() as ctx:
        resident = ctx.enter_context(tc.tile_pool(name="resident", bufs=1))
        qkv = ctx.enter_context(tc.tile_pool(name="qkv", bufs=1))

        ones128 = resident.tile([128, 128], BF)
        nc.vector.memset(ones128[:], 1.0)

        QT = qkv.tile([128, HL, S], BF)    # [HD, head, seq] (de-interleaved rows)
        KT = qkv.tile([128, KVL, S], BF)
        V = qkv.tile([128, NT, KVL * HD], BF)  # [seq%128, seqtile, kv-head*HD]

        # ---- stage 1: QKV projection + RoPE ----
        with tc.tile_pool(name="s1const", bufs=1) as s1const, \
             tc.tile_pool(name="xpool", bufs=2) as xpool, \
             tc.tile_pool(name="wpool", bufs=4) as wpool, \
             tc.tile_pool(name="tpool", bufs=3) as tpool, \
             tc.tile_pool(name="ps_qk", bufs=3, space="PSUM") as ps_qk, \
             tc.tile_pool(name="ps_w", bufs=2, space="PSUM") as ps_w, \
             tc.tile_pool(name="ps_v", bufs=2, space="PSUM") as ps_v:
            cosb = s1const.tile([128, S], F32)
            sinb = s1const.tile([128, S], F32)  # [-sin; +sin] halves
            wvb = s1const.tile([128, ND, KVL * HD], BF)
            # PE warm-up: dense ones@ones matmuls (no DMA dependency) keep
            # TensorE busy through the HAM window while the first x/weight
            # DMAs land, so real matmuls start at full clock.  Sized to end
            # just as chunk 0's first x block lands (idle would demote the
            # clock; excess would delay real work).
            for _ in range(64):
                wtile = ps_w.tile([128, 128], F32, tag="warm")
                nc.tensor.matmul(wtile[:], ones128[:], ones128[:],
                                 start=True, stop=True)

            def rope_emit(raw, f, js):
                # o = raw*cos + halfswap(raw)*sinN with no TensorE: the
                # half-swap is two partition-shifted ScalarE copies (same
                # engine as the evacuation, so ordering is free) and the
                # rotation sign lives in sinb = [-sin; +sin].
                rot = tpool.tile([128, 512], BF, tag="rot")
                nc.scalar.copy(out=rot[0:64, :], in_=raw[64:128, :])
                nc.scalar.copy(out=rot[64:128, :], in_=raw[0:64, :])
                t1 = tpool.tile([128, 512], F32, tag="t1")
                nc.vector.tensor_mul(t1[:], raw[:], cosb[:, js])
                t2 = tpool.tile([128, 512], F32, tag="t2")
                nc.vector.tensor_mul(t2[:], rot[:], sinb[:, js])
                dest = QT[:, f, js] if f < HL else KT[:, f - HL, js]
                nc.vector.tensor_add(dest, t1[:], t2[:])

            # The bulk x/cos/sin/wv stream rides the SP queue: the Sync
            # engine has no other stage-1 duties, so it can afford to stall
            # on DGE ring backpressure from many queued descriptors.
            # Weights ride the Act queue instead — never more than the
            # 3-deep prefetch window in flight, so ScalarE (which also runs
            # the PSUM evacuations) never blocks on a full ring.  GpSimd
            # issues instructions far too slowly to drive a queue.  Chunk 0
            # goes in 4-ktile blocks for progressive availability; chunks
            # 1-3 are single descriptors prefetched one chunk ahead.
            xtiles: dict = {}

            def xfetch(j):
                if j >= NJ or j in xtiles:
                    return
                xj = xpool.tile([128, ND, 512], BF, tag="x")
                if j == 0:
                    for blk in range(8):
                        nc.sync.dma_start(out=xj[:, 4 * blk:4 * blk + 4, :],
                                          in_=xt_d[:, j, 4 * blk:4 * blk + 4, :])
                else:
                    nc.sync.dma_start(out=xj[:], in_=xt_d[:, j])
                xtiles[j] = xj

            # weight prefetch runs a few tiles deep on its own (SP) queue
            n_groups = NJ * FQK
            wtiles: dict = {}
            wissued = 0

            def wprefetch(upto):
                nonlocal wissued
                while wissued < min(n_groups, upto):
                    wt = wpool.tile([128, ND, 128], BF, tag="wf")
                    # w0 rides the SP queue as its very first descriptor:
                    # the Act queue ramps slowly while SP hogs early HBM
                    # bandwidth, and w0 gates the first real matmul
                    eng = nc.sync if wissued == 0 else nc.scalar
                    eng.dma_start(out=wt[:], in_=wqk_d[wissued % FQK])
                    wtiles[wissued] = wt
                    wissued += 1

            wprefetch(3)  # w0 leads the SP queue; then chunk 0's x blocks
            xfetch(0)
            for j in range(NJ):
                js = bass.ts(j, 512)
                xj = xtiles.pop(j)
                for f in range(FQK):
                    gi = j * FQK + f
                    wf = wtiles.pop(gi)
                    # depth-4 window: the issue instruction sits in ScalarE's
                    # in-order stream ~one feature behind real time (behind
                    # evacuation waits), so a shallower window lets the PE
                    # catch up with the weight stream mid-chunk
                    wprefetch(gi + 4)
                    if j == 0 and f == 0:
                        # cos/sin full tensors behind chunk 0's x on the SP
                        # queue (a late sin only delays DVE-side rope, never
                        # the PE)
                        nc.sync.dma_start(out=cosb[:], in_=cos_d)
                        nc.sync.dma_start(out=sinb[:], in_=sin_d)
                    if j == 0 and f == 1:
                        nc.sync.dma_start(out=wvb[:], in_=wv_d[:])
                    if f == 4:
                        # prefetch next chunk's x while this chunk computes
                        # (behind wvb on the GpSimd queue for chunk 0)
                        xfetch(j + 1)
                    ps = ps_qk.tile([128, 512], F32, tag="qk")
                    for n in range(ND):
                        nc.tensor.matmul(ps[:], wf[:, n, :], xj[:, n, :],
                                         start=(n == 0), stop=(n == ND - 1))
                    raw = tpool.tile([128, 512], BF, tag="raw")
                    nc.scalar.copy(out=raw[:], in_=ps[:])
                    rope_emit(raw, f, js)
                for tt in range(4):
                    psv = ps_v.tile([128, KVL * HD], F32, tag="v")
                    for n in range(ND):
                        nc.tensor.matmul(psv[:], xj[:, n, bass.ts(tt, 128)],
                                         wvb[:, n, :],
                                         start=(n == 0), stop=(n == ND - 1))
                    nc.scalar.copy(out=V[:, j * 4 + tt, :], in_=psv[:])

        # attnT + wo live from stage 2 through stage 3 (pool opened only now
        # so stage 1 had the SBUF).
        att_out = ctx.enter_context(tc.tile_pool(name="att_out", bufs=1))
        attnT = att_out.tile([128, HL, S], BF)  # [HD, head, seq]
        wob = att_out.tile([128, HL, D], BF)

        # ---- stage 2+3: attention with interleaved output projection ----
        po_state = {"cur": None, "dd": 0, "drain": False, "alt": False,
                    "calt": False}

        def piece_atiles(pidx):
            q0, w = PIECES[pidx]
            if mask_mode == "zero":
                return q0, w, NT, []
            if mask_mode == "causal":
                nkt = (q0 + w) // 128
                return q0, w, nkt, list(range(q0 // 128, nkt))
            return q0, w, NT, list(range(NT))

        with tc.tile_pool(name="mpool", bufs=2 if mask_mode != "general" else 1) as mpool:
            pending_po = []  # (qt, nn) groups ready to emit as PE filler

            msk_tiles: dict = {}

            def load_msk(p):
                # mask tiles for piece p on the SP queue (issued one piece
                # ahead so they never gate a piece's first tensor_mul)
                if p >= len(PIECES) or p in msk_tiles:
                    return
                q0, w, nkt, atiles = piece_atiles(p)
                if not atiles:
                    msk_tiles[p] = None
                    return
                m = mpool.tile([128, len(atiles), w], BF, tag="msk")
                for idx, t in enumerate(atiles):
                    if mask_mode == "causal":
                        jj = t // 4
                        nc.sync.dma_start(
                            out=m[:, idx, :],
                            in_=mk_d[jj, t % 4][:, bass.ds(q0 - 512 * jj, w)])
                    else:
                        nc.sync.dma_start(
                            out=m[:, idx, :],
                            in_=mk_d[bass.ts(t, 128), bass.ds(q0, w)])
                msk_tiles[p] = m

            def run_piece(pidx, pools, pstep):
                ps_st, ps_o, ps_l, ppool, qpool, npool = pools
                q0, w, nkt, atiles = piece_atiles(pidx)
                js = bass.ds(q0, w)
                load_msk(pidx)  # no-op unless general mode (bufs=1, no prefetch)
                msk = msk_tiles.pop(pidx)
                if mask_mode != "general":
                    load_msk(pidx + 1)
                # k tiles are processed in PSUM-paired groups with one exp
                # per group — but only once po filler exists (pieces 0-1 are
                # latency-bound: coarser exp granularity exposes pipeline
                # latency the filler would otherwise cover)
                npair_max = 1 if pidx < 2 else (2 if w > 256 else 4)

                for h in range(HL):
                    hk = h // (HL // KVL)
                    outp = ps_o.tile([128, w], F32, tag="out")
                    lp = ps_l.tile([128, w], F32, tag="l")
                    pts = []
                    qsum = None
                    # software pipeline: PV_t is emitted one tile after QK_t
                    # so a full QK + filler sits in the PE stream while exp_t
                    # runs.  Diagonal tiles contribute nothing to masked
                    # columns, so PV runs only on the live sub-range.
                    def emit_pv(t):
                        off = max(0, 128 * t - q0) if mask_mode == "causal" else 0
                        nc.tensor.matmul(outp[:, off:w], V[:, t, bass.ts(hk, 128)],
                                         pts[t][:, off:w],
                                         start=(t == 0), stop=(t == nkt - 1),
                                         skip_group_check=True)

                    n_pv_done = 0
                    t = 0
                    while t < nkt:
                        npair = min(npair_max, nkt - t)
                        stp = ps_st.tile([128, npair, w], F32, tag="st")
                        pt2 = ppool.tile([128, npair, w], BF, tag="pt")
                        off0 = 0
                        for i in range(npair):
                            tt = t + i
                            # causal: columns q < 128t fully masked; compute
                            # QK only on the live sub-range.  Stale (finite)
                            # garbage in dead columns of paired tiles is
                            # exp'd then zeroed by the mask multiply (those
                            # PSUM slots have held bounded scores since the
                            # unpaired pieces 0-1 touched them full-width).
                            off = (max(0, 128 * tt - q0)
                                   if mask_mode == "causal" else 0)
                            if i == 0:
                                off0 = off
                            nc.tensor.matmul(stp[:, i, off:w],
                                             KT[:, hk, bass.ts(tt, 128)],
                                             QT[:, h, bass.ds(q0 + off, w - off)],
                                             start=True, stop=True)
                        if npair == 1 and off0 > 0:
                            # unpaired diagonal tile: exp only the live
                            # sub-range (never reads never-written PSUM)
                            nc.scalar.activation(
                                out=pt2[:, 0, off0:w], in_=stp[:, 0, off0:w],
                                func=mybir.ActivationFunctionType.Exp)
                        else:
                            nc.scalar.activation(
                                out=pt2[:], in_=stp[:],
                                func=mybir.ActivationFunctionType.Exp)
                        for i in range(npair):
                            tt = t + i
                            off_t = (max(0, 128 * tt - q0)
                                     if mask_mode == "causal" else 0)
                            if tt in atiles:
                                # multiplicative mask exp(m).  For causal
                                # masks only the 128-wide diagonal strip is
                                # partial — columns beyond it are all-ones
                                # and columns before it are skipped by every
                                # consumer, so the multiply (DVE) shrinks to
                                # the strip.
                                me = (min(off_t + 128, w)
                                      if mask_mode == "causal" else w)
                                mi = atiles.index(tt)
                                nc.vector.tensor_mul(
                                    pt2[:, i, off_t:me], pt2[:, i, off_t:me],
                                    msk[:, mi, off_t:me])
                            pts.append(pt2[:, i, :])
                            # running softmax-denominator sum in bf16 on DVE
                            # (live columns only); ONE broadcast matmul per
                            # (piece, head) at the end
                            if qsum is None:
                                qsum = qpool.tile([128, w], BF, tag="qs")
                                nc.vector.tensor_copy(qsum[:], pt2[:, i, :])
                            else:
                                nc.vector.tensor_add(qsum[:, off_t:w],
                                                     qsum[:, off_t:w],
                                                     pt2[:, i, off_t:w])
                        # wide pieces meter po filler so backlog survives
                        # into the tapered tail; narrow pieces drain harder
                        pstep({1: 1, 2: npair, 4: 6}[npair_max])
                        # PV lags one tile behind exp so a full QK + filler
                        # sits in the PE stream while exp runs
                        while n_pv_done < len(pts) - 1:
                            emit_pv(n_pv_done)
                            n_pv_done += 1
                        t += npair
                    nc.tensor.matmul(lp[:], ones128[:], qsum[:],
                                     start=True, stop=True)
                    while n_pv_done < nkt:
                        emit_pv(n_pv_done)
                        n_pv_done += 1
                    # fused evacuation + normalization on DVE (ScalarE stays
                    # exp-only; 1/l is a single fast-reciprocal op on the
                    # TensorE-broadcast denominator)
                    rcp = npool.tile([128, w], F32, tag="rcp")
                    nc.vector.reciprocal_approx_fast(out=rcp[:], in_=lp[:])
                    nc.vector.tensor_mul(attnT[:, h, js], outp[:], rcp[:])
                    # PE filler between heads covers the exp pipeline refill
                    pstep(16)
                pending_po.extend(
                    (qt, nn) for qt in range(q0 // 128, (q0 + w) // 128)
                    for nn in range(D // 512))

            def pstep_none(budget):
                return

            load_msk(0)
            if mask_mode != "general":  # bufs=1 pool: no prefetch
                load_msk(1)
            # wo heads 0-3 via the otherwise-idle GpSimd queue, heads 4-7
            # behind the first mask tiles on the SP queue; all land before
            # the po filler starts in piece 2.  Nothing rides the Act queue
            # here — its issuing engine (ScalarE) must stay free for exps.
            for dd in range(4):
                nc.gpsimd.dma_start(out=wob[:, dd, :], in_=wo_d[:, dd, :])
            for dd in range(4, HL):
                nc.sync.dma_start(out=wob[:, dd, :], in_=wo_d[:, dd, :])

            # block A — pieces 0-1, latency-bound, no filler available yet:
            # unpaired k tiles with a DEEP PSUM ring (4 score slots, double-
            # buffered output/denominator banks) so QK runs ahead of exp and
            # head boundaries never serialize on the evacuate chain.
            with tc.tile_pool(name="ppoolA", bufs=6) as ppoolA, \
                 tc.tile_pool(name="qpoolA", bufs=2) as qpoolA, \
                 tc.tile_pool(name="npoolA", bufs=2) as npoolA, \
                 tc.tile_pool(name="ps_stA", bufs=4, space="PSUM") as ps_stA, \
                 tc.tile_pool(name="ps_oA", bufs=2, space="PSUM") as ps_oA, \
                 tc.tile_pool(name="ps_lA", bufs=2, space="PSUM") as ps_lA:
                for pidx in (0, 1):
                    run_piece(pidx, (ps_stA, ps_oA, ps_lA, ppoolA, qpoolA,
                                     npoolA), pstep_none)

            # block B — pieces 2-4 + drain: paired exps + po filler
            with tc.tile_pool(name="ppool", bufs=8) as ppool, \
                 tc.tile_pool(name="qpool", bufs=2) as qpool, \
                 tc.tile_pool(name="npool", bufs=2) as npool, \
                 tc.tile_pool(name="spool", bufs=3) as spool, \
                 tc.tile_pool(name="ps_st", bufs=2, space="PSUM") as ps_st, \
                 tc.tile_pool(name="ps_o", bufs=1, space="PSUM") as ps_o, \
                 tc.tile_pool(name="ps_l", bufs=1, space="PSUM") as ps_l, \
                 tc.tile_pool(name="ps_po", bufs=2, space="PSUM") as ps_po:

                def po_step(budget):
                    # emit up to `budget` output-projection matmuls as PE
                    # filler; a group's PSUM accumulation legally interleaves
                    # with other banks' matmuls
                    for _ in range(budget):
                        if po_state["cur"] is None:
                            if not pending_po:
                                return
                            qt, nn = pending_po.pop(0)
                            pop = ps_po.tile([128, 512], F32, tag="po")
                            po_state["cur"] = (qt, nn, pop)
                            po_state["dd"] = 0
                        qt, nn, pop = po_state["cur"]
                        dd = po_state["dd"]
                        nc.tensor.matmul(pop[:], attnT[:, dd, bass.ts(qt, 128)],
                                         wob[:, dd, bass.ts(nn, 512)],
                                         start=(dd == 0), stop=(dd == HL - 1))
                        po_state["dd"] += 1
                        if po_state["dd"] == HL:
                            stg = spool.tile([128, 512], BF, tag="stg")
                            # in the drain ScalarE is exp-free: alternate the
                            # PSUM evacuation casts between DVE and ScalarE,
                            # and the output DMAs between the SP and Act
                            # HW-DGE queues, so neither tail-chains after the
                            # last matmuls
                            if po_state["drain"] and po_state["calt"]:
                                nc.scalar.copy(out=stg[:], in_=pop[:])
                            else:
                                nc.vector.tensor_copy(stg[:], pop[:])
                            po_state["calt"] = not po_state["calt"]
                            eng = nc.scalar if (po_state["drain"] and
                                                po_state["alt"]) else nc.sync
                            po_state["alt"] = not po_state["alt"]
                            eng.dma_start(
                                out=po_d[bass.ts(qt, 128), bass.ts(nn, 512)],
                                in_=stg[:])
                            po_state["cur"] = None

                for pidx in (2, 3, 4):
                    if pidx == len(PIECES) - 1:
                        po_state["drain"] = True
                    run_piece(pidx, (ps_st, ps_o, ps_l, ppool, qpool, npool),
                              po_step)
                po_state["drain"] = True
                while pending_po or po_state["cur"] is not None:
                    po_step(8)

    nc.compile()
    return nc


def _get_nc(mask_mode: str):
    if mask_mode not in _BUILD_CACHE:
        _BUILD_CACHE[mask_mode] = _build(mask_mode)
    return _BUILD_CACHE[mask_mode]


_DEINT = np.concatenate([np.arange(0, HD, 2), np.arange(1, HD, 2)])  # de-interleave


def _host_prep(x, freqs_cos, freqs_sin, mask, wq, wk, wv, wo):
    bf16 = ml_dtypes.bfloat16
    scale = float(HD) ** -0.5

    # mask mode
    mask = np.asarray(mask, np.float32)
    tril = np.tril(np.ones((S, S), bool))
    if np.all(mask == 0):
        mask_mode = "zero"
    elif np.all(mask[tril] == 0) and np.all(mask[~tril] <= -1e8):
        mask_mode = "causal"
    else:
        mask_mode = "general"

    # weights: de-interleave head dims of wq/wk; fold softmax scale into wq
    wq_p = (np.asarray(wq, np.float32).reshape(H, HD, D)[:, _DEINT, :] * scale)
    wk_p = np.asarray(wk, np.float32).reshape(KVH, HD, D)[:, _DEINT, :]
    wv_n = np.asarray(wv, np.float32).reshape(KVH, HD, D)
    wo_n = np.asarray(wo, np.float32)

    per_group = []
    for g in range(GROUPS):
        feats = np.concatenate([
            wq_p[g * HL:(g + 1) * HL].reshape(HL * HD, D),
            wk_p[g * KVL:(g + 1) * KVL].reshape(KVL * HD, D),
        ], axis=0)  # [1280, D]
        wqk_dma = np.ascontiguousarray(
            feats.reshape(FQK, 128, ND, 128).transpose(0, 3, 2, 1)).astype(bf16)
        wvg = wv_n[g * KVL:(g + 1) * KVL].reshape(KVL * HD, D)
        wv_dma = np.ascontiguousarray(
            wvg.reshape(KVL * HD, ND, 128).transpose(2, 1, 0)).astype(bf16)
        woT = wo_n[:, g * HL * HD:(g + 1) * HL * HD].T  # [1024, D]
        wo_dma = np.ascontiguousarray(
            woT.reshape(HL, 128, D).transpose(1, 0, 2)).astype(bf16)
        per_group.append((wqk_dma, wv_dma, wo_dma))

    xs = []
    for b in range(B):
        xT = np.asarray(x[b], np.float32).T  # [D, S]
        xs.append(np.ascontiguousarray(
            xT.reshape(ND, 128, NJ, 512).transpose(1, 2, 0, 3)).astype(bf16))

    cosT = np.asarray(freqs_cos, np.float32).T  # [64, S]
    sinT = np.asarray(freqs_sin, np.float32).T
    cos_dma = np.ascontiguousarray(np.concatenate([cosT, cosT], 0))
    # rotation sign folded into the sin table: o = raw*cos + halfswap(raw)*sinN
    sin_dma = np.ascontiguousarray(np.concatenate([-sinT, sinT], 0))

    # mask is applied multiplicatively after exp: P *= exp(mask)
    mask_extra = {}
    if mask_mode == "causal":
        mT = np.exp(np.minimum(mask.T, 0.0))
        md = np.empty((NJ, 4, 128, 512), np.float32)
        for j in range(NJ):
            for i in range(4):
                t = 4 * j + i
                md[j, i] = mT[t * 128:(t + 1) * 128, j * 512:(j + 1) * 512]
        mask_extra["maskd"] = md.astype(bf16)
    elif mask_mode == "general":
        with np.errstate(over="ignore"):
            mask_extra["maskt"] = np.ascontiguousarray(
                np.exp(mask.T)).astype(bf16)

    in_maps = []
    for c in range(N_CORES):
        b, g = c // GROUPS, c % GROUPS
        wqk_dma, wv_dma, wo_dma = per_group[g]
        m = {"xt": xs[b], "wqk": wqk_dma, "wv": wv_dma, "wo": wo_dma,
             "cosd": cos_dma, "sind": sin_dma}
        m.update(mask_extra)
        in_maps.append(m)
    return mask_mode, in_maps


def kernel(x, freqs_cos, freqs_sin, positions, mask, wq, wk, wv, wo,
           _want_profile=False):
    mask_mode, in_maps = _host_prep(x, freqs_cos, freqs_sin, mask, wq, wk, wv, wo)
    nc = _get_nc(mask_mode)
    res = run_bass_kernel_spmd(nc, in_maps, core_ids=list(range(N_CORES)),
                               trace=_want_profile)
    out = np.zeros((B, S, D), np.float32)
    for c in range(N_CORES):
        out[c // GROUPS] += np.asarray(res.results[c]["po"], np.float32)
    if _want_profile:
        kernel.last_exec_time_ns = res.exec_time_ns
        kernel.last_results = res
    return out
